# revision 9
# baseline (speedup 1.0000x reference)
"""Multi-head causal self-attention (B=4, T=2048, C=1024, H=16) on 8 TRN2 cores.

Sharding: core c handles batch b = c//2 and head-group hg = c%2 (8 heads):
data parallel over B, tensor parallel over H. Each core computes qkv^T for its
heads (x @ Wqkv column-slice, in transposed per-head-pair layout), causal
attention for its 8 heads, and a partial output projection (row-split W_proj)
-> y_partial [T, C]. Host: y[b] = y_partial[2b] + y_partial[2b+1] + b_proj.

All heavy matmuls run in float32r (TF32-like) on the PE array.
Scores are computed transposed ([k, q]) so softmax normalization sums arrive
via a ones-row folded into the attn@V matmul; normalization uses a K=1
broadcast matmul + vector multiply.
"""

from contextlib import ExitStack

import numpy as np

import concourse.bass as bass
import concourse.bacc as bacc
import concourse.mybir as mybir
import concourse.tile as tile
from concourse.bass_utils import run_bass_kernel_spmd
from concourse.masks import make_identity, make_upper_triangular

B, T, C, H, HS = 4, 2048, 1024, 16, 64
P = 128
NQC = T // 512          # q-chunks of 512
NKB = T // P            # key blocks of 128
NTB = T // P            # t blocks of 128
TH = T // 2             # t-half
SCALE = HS ** -0.5

F32 = mybir.dt.float32
F32R = mybir.dt.float32r
BF16 = mybir.dt.bfloat16
Exp = mybir.ActivationFunctionType.Exp


def build_kernel():
    nc = bacc.Bacc("TRN2", target_bir_lowering=False)

    x_d = nc.dram_tensor("x", (T, C), F32, kind="ExternalInput")
    wqkv_d = nc.dram_tensor("wqkv", (C, 12 * P), F32R, kind="ExternalInput")
    bqkv_d = nc.dram_tensor("bqkv", (12 * P,), F32, kind="ExternalInput")
    wproj_d = nc.dram_tensor("wproj", (8 * HS, C), F32R, kind="ExternalInput")
    y_d = nc.dram_tensor("y", (T, C), F32, kind="ExternalOutput")

    with tile.TileContext(nc) as tc, ExitStack() as big:
        const = big.enter_context(tc.tile_pool(name="const", bufs=1))
        persist = big.enter_context(tc.tile_pool(name="persist", bufs=1))

        # mask[k, q] = 1 where k <= q (valid causal entries of a diag block)
        mask = const.tile([P, P], F32, tag="mask")
        make_upper_triangular(nc, mask[:], val=1.0, diag=True)
        ones_f = const.tile([P, 64], F32, tag="ones_f")
        nc.vector.memset(ones_f[:], 1.0)
        ones_t = const.tile([1, 64], F32R, tag="ones")
        nc.vector.tensor_copy(ones_t[:], ones_f[0:1, :])

        # qk_all: 8 blocks of [128, T]; block 2p = qT of pair p, 2p+1 = kT.
        # Partitions 0:64 = head 2p, 64:128 = head 2p+1 (d on partitions).
        qk_all = persist.tile([P, 8 * T], BF16, tag="qk")
        # v_all: per (pair, kb): [vA(64) | onesA(1) | vB(64) | onesB(1)] = 130
        v_all = persist.tile([P, 4 * NKB * 130], F32R, tag="v")

        va4 = v_all[:].rearrange("p (a b c) -> p a b c", a=4, b=NKB, c=130)
        nc.vector.tensor_copy(va4[:, :, :, 64:65], ones_f[:, 0:4 * NKB])
        nc.vector.tensor_copy(va4[:, :, :, 129:130], ones_f[:, 0:4 * NKB])

        # ---------------- Phase 1: x^T, qkv^T, v transpose ----------------
        with ExitStack() as ph1:
            cp1 = ph1.enter_context(tc.tile_pool(name="cp1", bufs=1))
            xp = ph1.enter_context(tc.tile_pool(name="xp", bufs=2))
            xtp = ph1.enter_context(tc.tile_pool(name="xtp", bufs=1))
            wp = ph1.enter_context(tc.tile_pool(name="wp", bufs=2))
            vtp = ph1.enter_context(tc.tile_pool(name="vtp", bufs=2))
            ps_t = ph1.enter_context(tc.tile_pool(name="ps_t", bufs=3, space="PSUM"))
            ps_m = ph1.enter_context(tc.tile_pool(name="ps_m", bufs=3, space="PSUM"))

            ident = cp1.tile([P, P], F32, tag="ident")
            make_identity(nc, ident[:])
            bq = cp1.tile([P, 12], F32, tag="bq")
            nc.sync.dma_start(bq[:], bqkv_d[:].rearrange("(a p) -> p a", p=P))

            for th in range(2):  # t-half
                xT = xtp.tile([P, 8 * TH], F32R, tag="xT")
                for tb in range(8):
                    xt = xp.tile([P, C], F32, tag="x")
                    nc.sync.dma_start(
                        xt[:], x_d[th * TH + tb * P : th * TH + (tb + 1) * P, :]
                    )
                    for cb in range(8):
                        pt = ps_t.tile([P, P], F32, tag="pt")
                        nc.tensor.transpose(pt[:], xt[:, cb * P : (cb + 1) * P], ident[:])
                        nc.vector.tensor_copy(
                            xT[:, cb * TH + tb * P : cb * TH + (tb + 1) * P],
                            pt[:],
                        )
                for chb in range(12):
                    p_pair = chb // 3
                    kind = chb % 3
                    wb = wp.tile([P, 8 * P], F32R, tag="w")
                    nc.sync.dma_start(
                        wb[:].rearrange("p (cb j) -> p cb j", cb=8),
                        wqkv_d[:, chb * P : (chb + 1) * P].rearrange(
                            "(cb p) j -> p cb j", p=P
                        ),
                    )
                    for tck in range(2):
                        pq = ps_m.tile([P, 512], F32, tag="pq")
                        for cb in range(8):
                            nc.tensor.matmul(
                                pq[:],
                                wb[:, cb * P : (cb + 1) * P],
                                xT[:, cb * TH + tck * 512 : cb * TH + (tck + 1) * 512],
                                start=(cb == 0),
                                stop=(cb == 7),
                            )
                        t0 = th * TH + tck * 512
                        if kind < 2:  # q or k -> qk_all with bias
                            blk = p_pair * 2 + kind
                            nc.vector.tensor_scalar_add(
                                qk_all[:, blk * T + t0 : blk * T + t0 + 512],
                                pq[:],
                                bq[:, chb : chb + 1],
                            )
                        else:  # v: bias, then transpose to [t, d] layout
                            vt = vtp.tile([P, 512], F32, tag="vt")
                            nc.vector.tensor_scalar_add(vt[:], pq[:], bq[:, chb : chb + 1])
                            for tt in range(4):
                                kb = th * 8 + tck * 4 + tt
                                pt2 = ps_t.tile([P, P], F32, tag="pt")
                                nc.tensor.transpose(
                                    pt2[:], vt[:, tt * P : (tt + 1) * P], ident[:]
                                )
                                base = p_pair * NKB * 130 + kb * 130
                                dst = bass.AP(
                                    v_all[:].tensor,
                                    v_all[:].offset + base,
                                    [[v_all[:].ap[0][0], P], [65, 2], [1, 64]],
                                )
                                src = bass.AP(
                                    pt2[:].tensor,
                                    pt2[:].offset,
                                    [[pt2[:].ap[0][0], P], [64, 2], [1, 64]],
                                )
                                nc.vector.tensor_copy(dst, src)

        # aoT: pair-stacked [128 = ch(head 2p) | ch(head 2p+1), 4 * T]
        persist2 = big.enter_context(tc.tile_pool(name="persist2", bufs=1))
        aoT = persist2.tile([P, 4 * T], F32R, tag="aoT")

        # ---------------- Phase 2: attention ----------------
        with ExitStack() as ph2:
            atp = ph2.enter_context(tc.tile_pool(name="atp", bufs=2))
            rzp = ph2.enter_context(tc.tile_pool(name="rzp", bufs=2))
            bcsp = ph2.enter_context(tc.tile_pool(name="bcsp", bufs=2))
            stgp = ph2.enter_context(tc.tile_pool(name="stgp", bufs=3))
            ps_s = ph2.enter_context(tc.tile_pool(name="ps_s", bufs=3, space="PSUM"))
            ps_o = ph2.enter_context(tc.tile_pool(name="ps_o", bufs=1, space="PSUM"))
            ps_b = ph2.enter_context(tc.tile_pool(name="ps_b", bufs=1, space="PSUM"))

            for p_pair in range(4):
                qblk, kblk = 2 * p_pair, 2 * p_pair + 1
                for qc in range(NQC):
                    po0 = ps_o.tile([65, 512], F32, tag="po0")
                    po1 = ps_o.tile([65, 512], F32, tag="po1")
                    po = [po0, po1]
                    nkb = 4 * qc + 4
                    for kb in range(nkb):
                        qoff = max(0, kb * P - qc * 512)
                        for hh in range(2):
                            lo, hi = (0, 64) if hh == 0 else (64, P)
                            ps = ps_s.tile([P, 512], F32, tag="ps")
                            nc.tensor.matmul(
                                ps[:, qoff:512],
                                qk_all[lo:hi, kblk * T + kb * P : kblk * T + (kb + 1) * P],
                                qk_all[lo:hi, qblk * T + qc * 512 + qoff : qblk * T + (qc + 1) * 512],
                                start=True,
                                stop=True,
                            )
                            at = atp.tile([P, 512], F32R, tag=f"at{hh}")
                            nc.scalar.activation(
                                at[:, qoff:512], ps[:, qoff:512], Exp,
                                scale=SCALE,
                            )
                            if kb * P >= qc * 512:
                                # diagonal block: zero out k > q entries
                                nc.vector.tensor_mul(
                                    at[:, qoff : qoff + P],
                                    at[:, qoff : qoff + P],
                                    mask[:],
                                )
                            nc.tensor.matmul(
                                po[hh][:, qoff:512],
                                v_all[:, p_pair * NKB * 130 + kb * 130 + hh * 65 :
                                      p_pair * NKB * 130 + kb * 130 + hh * 65 + 65],
                                at[:, qoff:512],
                                start=(kb == 0),
                                stop=(kb == nkb - 1),
                                skip_group_check=True,
                            )
                    for hh in range(2):
                        rz = rzp.tile([1, 512], F32R, tag="rz")
                        with nc.allow_low_precision(reason="fp32r softmax denom"):
                            nc.vector.reciprocal(rz[:], po[hh][64:65, :])
                        pb = ps_b.tile([64, 512], F32, tag="pb")
                        nc.tensor.matmul(pb[:], ones_t[:], rz[:], start=True, stop=True)
                        bcs = bcsp.tile([64, 512], F32, tag="bcs")
                        nc.vector.tensor_copy(bcs[:], pb[:])
                        col = p_pair * T + qc * 512
                        if hh == 0:
                            nc.vector.tensor_mul(
                                aoT[0:64, col : col + 512],
                                po[hh][0:64, :],
                                bcs[:],
                            )
                        else:
                            stg = stgp.tile([64, 512], F32R, tag="stg")
                            nc.vector.tensor_mul(
                                stg[:], po[hh][0:64, :], bcs[:]
                            )
                            nc.sync.dma_start(aoT[64:P, col : col + 512], stg[:])

        # ---------------- Phase 3: output projection (partial) ----------------
        with ExitStack() as ph3:
            wpp = ph3.enter_context(tc.tile_pool(name="wpp", bufs=1))
            ysp = ph3.enter_context(tc.tile_pool(name="ysp", bufs=3))
            ps_y = ph3.enter_context(tc.tile_pool(name="ps_y", bufs=3, space="PSUM"))

            wpj = wpp.tile([P, 4 * C], F32R, tag="wpj")
            nc.sync.dma_start(
            wpj[:].rearrange("r (pr j) -> r pr j", pr=4),
            wproj_d[:].rearrange("(pr r) j -> r pr j", r=P),
        )

            for tb in range(NTB):
                for oc in range(2):
                    py = ps_y.tile([P, 512], F32, tag="py")
                    for pp in range(4):
                        nc.tensor.matmul(
                            py[:],
                            aoT[:, pp * T + tb * P : pp * T + (tb + 1) * P],
                            wpj[:, pp * C + oc * 512 : pp * C + (oc + 1) * 512],
                            start=(pp == 0),
                            stop=(pp == 3),
                        )
                    ys = ysp.tile([P, 512], F32, tag="ys")
                    nc.vector.tensor_copy(ys[:], py[:])
                    nc.sync.dma_start(
                        y_d[tb * P : (tb + 1) * P, oc * 512 : (oc + 1) * 512], ys[:]
                    )

    nc.compile()
    return nc


def _shard_inputs(x, W_qkv, b_qkv, W_proj):
    """Build the 8 per-core input maps."""
    in_maps = []
    for c in range(8):
        b = c // 2
        hg = c % 2
        heads = [hg * 8 + j for j in range(8)]
        cols = []
        for p in range(4):
            ha, hb = heads[2 * p], heads[2 * p + 1]
            for part in range(3):  # q, k, v
                cols.extend(range(ha * 192 + part * 64, ha * 192 + part * 64 + 64))
                cols.extend(range(hb * 192 + part * 64, hb * 192 + part * 64 + 64))
        cols = np.array(cols)
        in_maps.append(
            {
                "x": np.ascontiguousarray(x[b], dtype=np.float32),
                "wqkv": np.ascontiguousarray(W_qkv[:, cols], dtype=np.float32),
                "bqkv": np.ascontiguousarray(b_qkv[cols], dtype=np.float32),
                "wproj": np.ascontiguousarray(
                    W_proj[hg * 512 : (hg + 1) * 512, :], dtype=np.float32
                ),
            }
        )
    return in_maps


_NC = None


def kernel(x, W_qkv, b_qkv, W_proj, b_proj, _trace=False):
    global _NC
    x = np.asarray(x, dtype=np.float32)
    W_qkv = np.asarray(W_qkv, dtype=np.float32)
    b_qkv = np.asarray(b_qkv, dtype=np.float32)
    W_proj = np.asarray(W_proj, dtype=np.float32)
    b_proj = np.asarray(b_proj, dtype=np.float32)

    in_maps = _shard_inputs(x, W_qkv, b_qkv, W_proj)
    if _NC is None:
        _NC = build_kernel()
    res = run_bass_kernel_spmd(_NC, in_maps, core_ids=list(range(8)), trace=_trace)
    out = np.empty((B, T, C), dtype=np.float32)
    for b in range(B):
        out[b] = res.results[2 * b]["y"] + res.results[2 * b + 1]["y"] + b_proj
    if _trace:
        return out, res
    return out


# revision 11
# speedup vs baseline: 1.2856x; 1.2856x over previous
"""Multi-head causal self-attention (B=4, T=2048, C=1024, H=16) on 8 TRN2 cores.

Sharding: core c handles batch b = c//2 and head-group hg = c%2 (8 heads):
data parallel over B, tensor parallel over H. Each core computes qk^T for its
heads (xT @ Wqk column-slice, transposed per-head-pair layout), V in natural
layout, causal attention for its 8 heads, and a partial output projection
(row-split W_proj) -> y_partial [T, C]. Host transposes x per core and sums
y[b] = y_partial[2b] + y_partial[2b+1] + b_proj.

Matmul dtypes: q/k in bf16 (scores accumulate in fp32 PSUM), everything else
float32r (TF32-like). Scores are computed transposed ([k, q]) with
zero-padded q copies so every matmul is full-K at partition base 0. The
softmax denominator arrives via a ones-row folded into the attn@V matmul;
normalization uses a fast approximate reciprocal + K=1 broadcast matmul.
"""

from contextlib import ExitStack

import numpy as np

import concourse.bass as bass
import concourse.bacc as bacc
import concourse.mybir as mybir
import concourse.tile as tile
from concourse.bass_utils import run_bass_kernel_spmd
from concourse.masks import make_upper_triangular

B, T, C, H, HS = 4, 2048, 1024, 16, 64
P = 128
NQC = T // 512          # q-chunks of 512
NKB = T // P            # key blocks of 128
NTB = T // P            # t blocks of 128
TH = T // 2             # t-half
SCALE = HS ** -0.5

F32 = mybir.dt.float32
F32R = mybir.dt.float32r
BF16 = mybir.dt.bfloat16
Exp = mybir.ActivationFunctionType.Exp


def build_kernel():
    nc = bacc.Bacc("TRN2", target_bir_lowering=False)

    xt_d = nc.dram_tensor("xt", (C, T), F32R, kind="ExternalInput")
    wqk_d = nc.dram_tensor("wqk", (C, 8 * P), F32R, kind="ExternalInput")
    bqk_d = nc.dram_tensor("bqk", (8 * P,), F32, kind="ExternalInput")
    wv_d = nc.dram_tensor("wv", (C, 512), F32R, kind="ExternalInput")
    bv_d = nc.dram_tensor("bv", (1, 512), F32R, kind="ExternalInput")
    wproj_d = nc.dram_tensor("wproj", (8 * HS, C), F32R, kind="ExternalInput")
    y_d = nc.dram_tensor("y", (T, C), F32, kind="ExternalOutput")

    with tile.TileContext(nc) as tc, ExitStack() as big:
        const = big.enter_context(tc.tile_pool(name="const", bufs=1))
        persist = big.enter_context(tc.tile_pool(name="persist", bufs=1))

        # mask[k, q] = 1 where k <= q (valid causal entries of a diag block)
        mask = const.tile([P, P], F32, tag="mask")
        make_upper_triangular(nc, mask[:], val=1.0, diag=True)
        ones_f = const.tile([P, P], F32, tag="ones_f")
        nc.vector.memset(ones_f[:], 1.0)
        ones_t = const.tile([1, P], F32R, tag="ones")
        nc.vector.tensor_copy(ones_t[:], ones_f[0:1, :])

        # qk_all: 12 blocks of [128, T] bf16; per pair p:
        #   block 3p   = qpadA: rows 0:64 q of head 2p, rows 64:128 zero
        #   block 3p+1 = qpadB: rows 0:64 zero, rows 64:128 q of head 2p+1
        #   block 3p+2 = k pair: rows 0:64 k(2p), 64:128 k(2p+1)
        qk_all = persist.tile([P, 12 * T], BF16, tag="qk")
        for p_pair in range(4):
            nc.vector.memset(qk_all[64:P, (3 * p_pair) * T : (3 * p_pair + 1) * T], 0.0)
            nc.vector.memset(qk_all[0:64, (3 * p_pair + 1) * T : (3 * p_pair + 2) * T], 0.0)

        # v_all: per (pair, kb): [vA(64) | onesA(1) | vB(64) | onesB(1)] = 130
        v_all = persist.tile([P, 4 * NKB * 130], F32R, tag="v")
        va4 = v_all[:].rearrange("p (a b c) -> p a b c", a=4, b=NKB, c=130)
        nc.vector.tensor_copy(va4[:, :, :, 64:65], ones_f[:, 0 : 4 * NKB])
        nc.vector.tensor_copy(va4[:, :, :, 129:130], ones_f[:, 0 : 4 * NKB])

        # ---------------- Phase 1: qk^T, natural V ----------------
        with ExitStack() as ph1:
            cp1 = ph1.enter_context(tc.tile_pool(name="cp1", bufs=1))
            xtp = ph1.enter_context(tc.tile_pool(name="xtp", bufs=1))
            wp = ph1.enter_context(tc.tile_pool(name="wp", bufs=2))
            ps_m = ph1.enter_context(tc.tile_pool(name="ps_m", bufs=3, space="PSUM"))
            ps_v = ph1.enter_context(tc.tile_pool(name="ps_v", bufs=3, space="PSUM"))
            ps_c = ph1.enter_context(tc.tile_pool(name="ps_c", bufs=1, space="PSUM"))

            bqk = cp1.tile([P, 8], F32, tag="bqk")
            nc.sync.dma_start(bqk[:], bqk_d[:].rearrange("(a p) -> p a", p=P))
            wv_sb = cp1.tile([P, 8 * 512], F32R, tag="wv")
            nc.sync.dma_start(
                wv_sb[:].rearrange("p (cb j) -> p cb j", cb=8),
                wv_d[:].rearrange("(cb p) j -> p cb j", p=P),
            )
            # bias_v[128, 512] = b_v broadcast along partitions (K=1 matmul)
            bvr = cp1.tile([1, 512], F32R, tag="bvr")
            nc.sync.dma_start(bvr[:], bv_d[:])
            bias_v = cp1.tile([P, 512], F32, tag="bias_v")
            pbv = ps_c.tile([P, 512], F32, tag="pbv")
            nc.tensor.matmul(pbv[:], ones_t[:], bvr[:], start=True, stop=True)
            nc.vector.tensor_copy(bias_v[:], pbv[:])

            for th in range(2):  # t-half
                xT = xtp.tile([P, 8 * TH], F32R, tag="xT")
                for cb in range(8):
                    nc.sync.dma_start(
                        xT[:, cb * TH : (cb + 1) * TH],
                        xt_d[cb * P : (cb + 1) * P, th * TH : (th + 1) * TH],
                    )
                # q, k projections (transposed layout)
                for chb in range(8):
                    p_pair = chb // 2
                    kind = chb % 2  # 0 = q block, 1 = k block
                    wb = wp.tile([P, 8 * P], F32R, tag="w")
                    nc.sync.dma_start(
                        wb[:].rearrange("p (cb j) -> p cb j", cb=8),
                        wqk_d[:, chb * P : (chb + 1) * P].rearrange(
                            "(cb p) j -> p cb j", p=P
                        ),
                    )
                    for tck in range(2):
                        pq = ps_m.tile([P, 512], F32, tag="pq")
                        for cb in range(8):
                            nc.tensor.matmul(
                                pq[:],
                                wb[:, cb * P : (cb + 1) * P],
                                xT[:, cb * TH + tck * 512 : cb * TH + (tck + 1) * 512],
                                start=(cb == 0),
                                stop=(cb == 7),
                            )
                        t0 = th * TH + tck * 512
                        if kind == 0:  # q -> two zero-padded tiles
                            blk_a, blk_b = 3 * p_pair, 3 * p_pair + 1
                            nc.vector.tensor_scalar_add(
                                qk_all[0:64, blk_a * T + t0 : blk_a * T + t0 + 512],
                                pq[0:64, :],
                                bqk[0:64, chb : chb + 1],
                            )
                            nc.vector.tensor_scalar_add(
                                qk_all[64:P, blk_b * T + t0 : blk_b * T + t0 + 512],
                                pq[64:P, :],
                                bqk[64:P, chb : chb + 1],
                            )
                        else:  # k pair block
                            blk = 3 * p_pair + 2
                            nc.vector.tensor_scalar_add(
                                qk_all[:, blk * T + t0 : blk * T + t0 + 512],
                                pq[:],
                                bqk[:, chb : chb + 1],
                            )
                # natural-layout V (xT stationary, wv moving)
                for tb in range(8):
                    kb = th * 8 + tb
                    pv = ps_v.tile([P, 512], F32, tag="pv")
                    for cb in range(8):
                        nc.tensor.matmul(
                            pv[:],
                            xT[:, cb * TH + tb * P : cb * TH + (tb + 1) * P],
                            wv_sb[:, cb * 512 : (cb + 1) * 512],
                            start=(cb == 0),
                            stop=(cb == 7),
                        )
                    dst = bass.AP(
                        v_all[:].tensor,
                        v_all[:].offset + kb * 130,
                        [[v_all[:].ap[0][0], P], [NKB * 130, 4], [65, 2], [1, 64]],
                    )
                    src = bass.AP(
                        pv[:].tensor,
                        pv[:].offset,
                        [[pv[:].ap[0][0], P], [128, 4], [64, 2], [1, 64]],
                    )
                    bsrc = bass.AP(
                        bias_v[:].tensor,
                        bias_v[:].offset,
                        [[bias_v[:].ap[0][0], P], [128, 4], [64, 2], [1, 64]],
                    )
                    nc.vector.tensor_tensor(dst, src, bsrc, mybir.AluOpType.add)

        # aoT: pair-stacked [128 = ch(head 2p) | ch(head 2p+1), 4 * T]
        persist2 = big.enter_context(tc.tile_pool(name="persist2", bufs=1))
        aoT = persist2.tile([P, 4 * T], F32R, tag="aoT")

        # ---------------- Phase 2: attention ----------------
        with ExitStack() as ph2:
            atp = ph2.enter_context(tc.tile_pool(name="atp", bufs=2))
            rzp = ph2.enter_context(tc.tile_pool(name="rzp", bufs=2))
            bcsp = ph2.enter_context(tc.tile_pool(name="bcsp", bufs=2))
            stgp = ph2.enter_context(tc.tile_pool(name="stgp", bufs=3))
            ps_s = ph2.enter_context(tc.tile_pool(name="ps_s", bufs=3, space="PSUM"))
            ps_o = ph2.enter_context(tc.tile_pool(name="ps_o", bufs=2, space="PSUM"))
            ps_b = ph2.enter_context(tc.tile_pool(name="ps_b", bufs=1, space="PSUM"))

            for p_pair in range(4):
                kblk = 3 * p_pair + 2
                for qc in range(NQC):
                    po0 = ps_o.tile([65, 512], F32, tag="po0")
                    po1 = ps_o.tile([65, 512], F32, tag="po1")
                    po = [po0, po1]
                    nkb = 4 * qc + 4
                    for kb in range(nkb):
                        qoff = max(0, kb * P - qc * 512)
                        for hh in range(2):
                            qblk = 3 * p_pair + hh
                            ps = ps_s.tile([P, 512], F32, tag="ps")
                            nc.tensor.matmul(
                                ps[:, qoff:512],
                                qk_all[:, kblk * T + kb * P : kblk * T + (kb + 1) * P],
                                qk_all[:, qblk * T + qc * 512 + qoff : qblk * T + (qc + 1) * 512],
                                start=True,
                                stop=True,
                            )
                            at = atp.tile([P, 512], F32R, tag=f"at{hh}")
                            nc.scalar.activation(
                                at[:, qoff:512], ps[:, qoff:512], Exp, scale=SCALE,
                            )
                            if kb * P >= qc * 512:
                                # diagonal block: zero out k > q entries
                                nc.vector.tensor_mul(
                                    at[:, qoff : qoff + P],
                                    at[:, qoff : qoff + P],
                                    mask[:],
                                )
                            nc.tensor.matmul(
                                po[hh][:, qoff:512],
                                v_all[:, p_pair * NKB * 130 + kb * 130 + hh * 65 :
                                      p_pair * NKB * 130 + kb * 130 + hh * 65 + 65],
                                at[:, qoff:512],
                                start=(kb == 0),
                                stop=(kb == nkb - 1),
                                skip_group_check=True,
                            )
                    for hh in range(2):
                        rz = rzp.tile([1, 512], F32R, tag="rz")
                        with nc.allow_low_precision(reason="fp32r softmax denom"):
                            nc.vector.reciprocal(rz[:], po[hh][64:65, :])
                        pb = ps_b.tile([64, 512], F32, tag="pb")
                        nc.tensor.matmul(
                            pb[:], ones_t[:, 0:64], rz[:], start=True, stop=True
                        )
                        bcs = bcsp.tile([64, 512], F32, tag="bcs")
                        nc.vector.tensor_copy(bcs[:], pb[:])
                        col = p_pair * T + qc * 512
                        if hh == 0:
                            nc.vector.tensor_mul(
                                aoT[0:64, col : col + 512], po[hh][0:64, :], bcs[:],
                            )
                        else:
                            stg = stgp.tile([64, 512], F32R, tag="stg")
                            nc.vector.tensor_mul(stg[:], po[hh][0:64, :], bcs[:])
                            nc.sync.dma_start(aoT[64:P, col : col + 512], stg[:])

        # ---------------- Phase 3: output projection (partial) ----------------
        with ExitStack() as ph3:
            wpp = ph3.enter_context(tc.tile_pool(name="wpp", bufs=1))
            ysp = ph3.enter_context(tc.tile_pool(name="ysp", bufs=3))
            ps_y = ph3.enter_context(tc.tile_pool(name="ps_y", bufs=3, space="PSUM"))

            wpj = wpp.tile([P, 4 * C], F32R, tag="wpj")
            nc.sync.dma_start(
                wpj[:].rearrange("r (pr j) -> r pr j", pr=4),
                wproj_d[:].rearrange("(pr r) j -> r pr j", r=P),
            )

            for tb in range(NTB):
                for oc in range(2):
                    py = ps_y.tile([P, 512], F32, tag="py")
                    for pp in range(4):
                        nc.tensor.matmul(
                            py[:],
                            aoT[:, pp * T + tb * P : pp * T + (tb + 1) * P],
                            wpj[:, pp * C + oc * 512 : pp * C + (oc + 1) * 512],
                            start=(pp == 0),
                            stop=(pp == 3),
                        )
                    ys = ysp.tile([P, 512], F32, tag="ys")
                    nc.vector.tensor_copy(ys[:], py[:])
                    nc.sync.dma_start(
                        y_d[tb * P : (tb + 1) * P, oc * 512 : (oc + 1) * 512], ys[:]
                    )

    nc.compile()
    return nc


def _shard_inputs(x, W_qkv, b_qkv, W_proj):
    """Build the 8 per-core input maps."""
    in_maps = []
    for c in range(8):
        b = c // 2
        hg = c % 2
        heads = [hg * 8 + j for j in range(8)]
        qk_cols = []
        for p in range(4):
            ha, hb = heads[2 * p], heads[2 * p + 1]
            for part in range(2):  # q, k
                qk_cols.extend(range(ha * 192 + part * 64, ha * 192 + part * 64 + 64))
                qk_cols.extend(range(hb * 192 + part * 64, hb * 192 + part * 64 + 64))
        qk_cols = np.array(qk_cols)
        v_cols = []
        for p in range(4):
            ha, hb = heads[2 * p], heads[2 * p + 1]
            v_cols.extend(range(ha * 192 + 128, ha * 192 + 192))
            v_cols.extend(range(hb * 192 + 128, hb * 192 + 192))
        v_cols = np.array(v_cols)
        in_maps.append(
            {
                "xt": np.ascontiguousarray(x[b].T, dtype=np.float32),
                "wqk": np.ascontiguousarray(W_qkv[:, qk_cols], dtype=np.float32),
                "bqk": np.ascontiguousarray(b_qkv[qk_cols], dtype=np.float32),
                "wv": np.ascontiguousarray(W_qkv[:, v_cols], dtype=np.float32),
                "bv": np.ascontiguousarray(
                    b_qkv[v_cols].reshape(1, 512), dtype=np.float32
                ),
                "wproj": np.ascontiguousarray(
                    W_proj[hg * 512 : (hg + 1) * 512, :], dtype=np.float32
                ),
            }
        )
    return in_maps


_NC = None


def kernel(x, W_qkv, b_qkv, W_proj, b_proj, _trace=False):
    global _NC
    x = np.asarray(x, dtype=np.float32)
    W_qkv = np.asarray(W_qkv, dtype=np.float32)
    b_qkv = np.asarray(b_qkv, dtype=np.float32)
    W_proj = np.asarray(W_proj, dtype=np.float32)
    b_proj = np.asarray(b_proj, dtype=np.float32)

    in_maps = _shard_inputs(x, W_qkv, b_qkv, W_proj)
    if _NC is None:
        _NC = build_kernel()
    res = run_bass_kernel_spmd(
        _NC, in_maps, core_ids=list(range(8)), trace=_trace,
        trace_cores=list(range(8)) if _trace else None,
    )
    out = np.empty((B, T, C), dtype=np.float32)
    for b in range(B):
        out[b] = res.results[2 * b]["y"] + res.results[2 * b + 1]["y"] + b_proj
    if _trace:
        return out, res
    return out


# revision 15
# speedup vs baseline: 1.3385x; 1.0412x over previous
"""Multi-head causal self-attention (B=4, T=2048, C=1024, H=16) on 8 TRN2 cores.

Sharding: core c handles batch b = c//2 and head-group hg = c%2 (8 heads):
data parallel over B, tensor parallel over H. Each core computes qk^T for its
heads (xT @ Wqk column-slice, transposed per-head-pair layout), V in natural
layout, causal attention for its 8 heads, and a partial output projection
(row-split W_proj) -> y_partial [T, C]. Host transposes x per core and sums
y[b] = y_partial[2b] + y_partial[2b+1] + b_proj.

Matmul dtypes: q/k in bf16 (scores accumulate in fp32 PSUM), everything else
float32r (TF32-like). Scores are computed transposed ([k, q]) with
zero-padded q copies so every matmul is full-K at partition base 0. The
softmax denominator arrives via a ones-row folded into the attn@V matmul;
normalization uses a fast approximate reciprocal + K=1 broadcast matmul.
"""

from contextlib import ExitStack

import numpy as np

import concourse.bass as bass
import concourse.bacc as bacc
import concourse.mybir as mybir
import concourse.tile as tile
from concourse.bass_utils import run_bass_kernel_spmd
from concourse.masks import make_upper_triangular

B, T, C, H, HS = 4, 2048, 1024, 16, 64
P = 128
NQC = T // 512          # q-chunks of 512
NKB = T // P            # key blocks of 128
NTB = T // P            # t blocks of 128
TH = T // 2             # t-half
SCALE = HS ** -0.5

F32 = mybir.dt.float32
F32R = mybir.dt.float32r
BF16 = mybir.dt.bfloat16
Exp = mybir.ActivationFunctionType.Exp


def build_kernel():
    nc = bacc.Bacc("TRN2", target_bir_lowering=False)

    xt_d = nc.dram_tensor("xt", (C, T), F32R, kind="ExternalInput")
    wqk_d = nc.dram_tensor("wqk", (C, 8 * P), F32R, kind="ExternalInput")
    bqk_d = nc.dram_tensor("bqk", (8 * P,), F32, kind="ExternalInput")
    wv_d = nc.dram_tensor("wv", (C, 512), F32R, kind="ExternalInput")
    bv_d = nc.dram_tensor("bv", (1, 512), F32R, kind="ExternalInput")
    wproj_d = nc.dram_tensor("wproj", (8 * HS, C), F32R, kind="ExternalInput")
    y_d = nc.dram_tensor("y", (T, C), F32, kind="ExternalOutput")

    with tile.TileContext(nc) as tc, ExitStack() as big:
        const = big.enter_context(tc.tile_pool(name="const", bufs=1))
        persist = big.enter_context(tc.tile_pool(name="persist", bufs=1))

        # mask[k, q] = 1 where k <= q (valid causal entries of a diag block)
        mask = const.tile([P, P], F32, tag="mask")
        make_upper_triangular(nc, mask[:], val=1.0, diag=True)
        ones_f = const.tile([P, P], F32, tag="ones_f")
        nc.vector.memset(ones_f[:], 1.0)
        ones_t = const.tile([1, P], F32R, tag="ones")
        nc.vector.tensor_copy(ones_t[:], ones_f[0:1, :])

        # qk_all: 12 blocks of [128, T] bf16; per pair p:
        #   block 3p   = qpadA: rows 0:64 q of head 2p, rows 64:128 zero
        #   block 3p+1 = qpadB: rows 0:64 zero, rows 64:128 q of head 2p+1
        #   block 3p+2 = k pair: rows 0:64 k(2p), 64:128 k(2p+1)
        qk_all = persist.tile([P, 12 * T], BF16, tag="qk")
        for p_pair in range(4):
            nc.vector.memset(qk_all[64:P, (3 * p_pair) * T : (3 * p_pair + 1) * T], 0.0)
            nc.vector.memset(qk_all[0:64, (3 * p_pair + 1) * T : (3 * p_pair + 2) * T], 0.0)

        # v_all: per (pair, kb): [vA(64) | onesA(1) | vB(64) | onesB(1)] = 130
        v_all = persist.tile([P, 4 * NKB * 130], F32R, tag="v")
        va4 = v_all[:].rearrange("p (a b c) -> p a b c", a=4, b=NKB, c=130)
        nc.vector.tensor_copy(va4[:, :, :, 64:65], ones_f[:, 0 : 4 * NKB])
        nc.vector.tensor_copy(va4[:, :, :, 129:130], ones_f[:, 0 : 4 * NKB])

        # ---------------- Phase 1: qk^T, natural V ----------------
        with ExitStack() as ph1:
            cp1 = ph1.enter_context(tc.tile_pool(name="cp1", bufs=1))
            xtp = ph1.enter_context(tc.tile_pool(name="xtp", bufs=1))
            wp = ph1.enter_context(tc.tile_pool(name="wp", bufs=2))
            ps_m = ph1.enter_context(tc.tile_pool(name="ps_m", bufs=3, space="PSUM"))
            ps_v = ph1.enter_context(tc.tile_pool(name="ps_v", bufs=3, space="PSUM"))
            ps_c = ph1.enter_context(tc.tile_pool(name="ps_c", bufs=1, space="PSUM"))

            bqk = cp1.tile([P, 8], F32, tag="bqk")
            nc.sync.dma_start(bqk[:], bqk_d[:].rearrange("(a p) -> p a", p=P))
            wv_sb = cp1.tile([P, 8 * 512], F32R, tag="wv")
            nc.sync.dma_start(
                wv_sb[:].rearrange("p (cb j) -> p cb j", cb=8),
                wv_d[:].rearrange("(cb p) j -> p cb j", p=P),
            )
            # bias_v[128, 512] = b_v broadcast along partitions (K=1 matmul)
            bvr = cp1.tile([1, 512], F32R, tag="bvr")
            nc.sync.dma_start(bvr[:], bv_d[:])
            bias_v = cp1.tile([P, 512], F32, tag="bias_v")
            pbv = ps_c.tile([P, 512], F32, tag="pbv")
            nc.tensor.matmul(pbv[:], ones_t[:], bvr[:], start=True, stop=True)
            nc.vector.tensor_copy(bias_v[:], pbv[:])

            for th in range(2):  # t-half
                xT = xtp.tile([P, 8 * TH], F32R, tag="xT")
                for cb in range(8):
                    nc.sync.dma_start(
                        xT[:, cb * TH : (cb + 1) * TH],
                        xt_d[cb * P : (cb + 1) * P, th * TH : (th + 1) * TH],
                    )
                # q, k projections (transposed layout)
                for chb in range(8):
                    p_pair = chb // 2
                    kind = chb % 2  # 0 = q block, 1 = k block
                    wb = wp.tile([P, 8 * P], F32R, tag="w")
                    nc.sync.dma_start(
                        wb[:].rearrange("p (cb j) -> p cb j", cb=8),
                        wqk_d[:, chb * P : (chb + 1) * P].rearrange(
                            "(cb p) j -> p cb j", p=P
                        ),
                    )
                    for tck in range(2):
                        pq = ps_m.tile([P, 512], F32, tag="pq")
                        for cb in range(8):
                            nc.tensor.matmul(
                                pq[:],
                                wb[:, cb * P : (cb + 1) * P],
                                xT[:, cb * TH + tck * 512 : cb * TH + (tck + 1) * 512],
                                start=(cb == 0),
                                stop=(cb == 7),
                            )
                        t0 = th * TH + tck * 512
                        if kind == 0:  # q -> two zero-padded tiles
                            blk_a, blk_b = 3 * p_pair, 3 * p_pair + 1
                            nc.vector.tensor_scalar_add(
                                qk_all[0:64, blk_a * T + t0 : blk_a * T + t0 + 512],
                                pq[0:64, :],
                                bqk[0:64, chb : chb + 1],
                            )
                            nc.vector.tensor_scalar_add(
                                qk_all[64:P, blk_b * T + t0 : blk_b * T + t0 + 512],
                                pq[64:P, :],
                                bqk[64:P, chb : chb + 1],
                            )
                        else:  # k pair block
                            blk = 3 * p_pair + 2
                            nc.vector.tensor_scalar_add(
                                qk_all[:, blk * T + t0 : blk * T + t0 + 512],
                                pq[:],
                                bqk[:, chb : chb + 1],
                            )
                # natural-layout V (xT stationary, wv moving)
                for tb in range(8):
                    kb = th * 8 + tb
                    pv = ps_v.tile([P, 512], F32, tag="pv")
                    for cb in range(8):
                        nc.tensor.matmul(
                            pv[:],
                            xT[:, cb * TH + tb * P : cb * TH + (tb + 1) * P],
                            wv_sb[:, cb * 512 : (cb + 1) * 512],
                            start=(cb == 0),
                            stop=(cb == 7),
                        )
                    dst = bass.AP(
                        v_all[:].tensor,
                        v_all[:].offset + kb * 130,
                        [[v_all[:].ap[0][0], P], [NKB * 130, 4], [65, 2], [1, 64]],
                    )
                    src = bass.AP(
                        pv[:].tensor,
                        pv[:].offset,
                        [[pv[:].ap[0][0], P], [128, 4], [64, 2], [1, 64]],
                    )
                    bsrc = bass.AP(
                        bias_v[:].tensor,
                        bias_v[:].offset,
                        [[bias_v[:].ap[0][0], P], [128, 4], [64, 2], [1, 64]],
                    )
                    nc.vector.tensor_tensor(dst, src, bsrc, mybir.AluOpType.add)

        # aoT: pair-stacked [128 = ch(head 2p) | ch(head 2p+1), 4 * T]
        persist2 = big.enter_context(tc.tile_pool(name="persist2", bufs=1))
        aoT = persist2.tile([P, 4 * T], F32R, tag="aoT")

        # wproj prefetch (DMA overlaps with attention)
        wpp = big.enter_context(tc.tile_pool(name="wpp", bufs=1))
        wpj = wpp.tile([P, 4 * C], F32R, tag="wpj")
        nc.sync.dma_start(
            wpj[:].rearrange("r (pr j) -> r pr j", pr=4),
            wproj_d[:].rearrange("(pr r) j -> r pr j", r=P),
        )

        # ---------------- Phase 2: attention ----------------
        with ExitStack() as ph2:
            atp = ph2.enter_context(tc.tile_pool(name="atp", bufs=2))
            rzp = ph2.enter_context(tc.tile_pool(name="rzp", bufs=2))
            bcsp = ph2.enter_context(tc.tile_pool(name="bcsp", bufs=2))
            stgp = ph2.enter_context(tc.tile_pool(name="stgp", bufs=3))
            ps_s = ph2.enter_context(tc.tile_pool(name="ps_s", bufs=3, space="PSUM"))
            ps_o = ph2.enter_context(tc.tile_pool(name="ps_o", bufs=2, space="PSUM"))
            ps_b = ph2.enter_context(tc.tile_pool(name="ps_b", bufs=1, space="PSUM"))

            def emit_tail(p_pair, qc, po):
                for hh in range(2):
                    rzf = rzp.tile([65, 512], F32, tag="rzf")
                    nc.vector.reciprocal(rzf[64:65, :], po[hh][64:65, :])
                    pb = ps_b.tile([64, 512], F32, tag="pb")
                    nc.tensor.matmul(
                        pb[:], ones_f[64:65, 0:64], rzf[64:65, :],
                        start=True, stop=True,
                    )
                    bcs = bcsp.tile([64, 512], F32, tag="bcs")
                    nc.vector.tensor_copy(bcs[:], pb[:])
                    col = p_pair * T + qc * 512
                    if hh == 0:
                        nc.vector.tensor_mul(
                            aoT[0:64, col : col + 512], po[hh][0:64, :], bcs[:],
                        )
                    else:
                        stg = stgp.tile([64, 512], F32R, tag="stg")
                        nc.vector.tensor_mul(stg[:], po[hh][0:64, :], bcs[:])
                        nc.sync.dma_start(aoT[64:P, col : col + 512], stg[:])

            pending = []
            for p_pair in range(4):
                kblk = 3 * p_pair + 2
                for qc in range(NQC):
                    po0 = ps_o.tile([65, 512], F32, tag="po0")
                    po1 = ps_o.tile([65, 512], F32, tag="po1")
                    po = [po0, po1]
                    nkb = 4 * qc + 4
                    for kb in range(nkb):
                        qoff = max(0, kb * P - qc * 512)
                        for hh in range(2):
                            qblk = 3 * p_pair + hh
                            ps = ps_s.tile([P, 512], F32, tag="ps")
                            nc.tensor.matmul(
                                ps[:, qoff:512],
                                qk_all[:, kblk * T + kb * P : kblk * T + (kb + 1) * P],
                                qk_all[:, qblk * T + qc * 512 + qoff : qblk * T + (qc + 1) * 512],
                                start=True,
                                stop=True,
                            )
                            at = atp.tile([P, 512], F32R, tag=f"at{hh}")
                            nc.scalar.activation(
                                at[:, qoff:512], ps[:, qoff:512], Exp, scale=SCALE,
                            )
                            if kb * P >= qc * 512:
                                # diagonal block: zero out k > q entries
                                nc.vector.tensor_mul(
                                    at[:, qoff : qoff + P],
                                    at[:, qoff : qoff + P],
                                    mask[:],
                                )
                            nc.tensor.matmul(
                                po[hh][:, qoff:512],
                                v_all[:, p_pair * NKB * 130 + kb * 130 + hh * 65 :
                                      p_pair * NKB * 130 + kb * 130 + hh * 65 + 65],
                                at[:, qoff:512],
                                start=(kb == 0),
                                stop=(kb == nkb - 1),
                                skip_group_check=True,
                            )
                    pending.append((p_pair, qc, po))
                    if len(pending) >= 2:
                        emit_tail(*pending.pop(0))
            for args in pending:
                emit_tail(*args)

        # ---------------- Phase 3: output projection (partial) ----------------
        with ExitStack() as ph3:
            ysp = ph3.enter_context(tc.tile_pool(name="ysp", bufs=3))
            ps_y = ph3.enter_context(tc.tile_pool(name="ps_y", bufs=3, space="PSUM"))

            for tb in range(NTB):
                for oc in range(2):
                    py = ps_y.tile([P, 512], F32, tag="py")
                    for pp in range(4):
                        nc.tensor.matmul(
                            py[:],
                            aoT[:, pp * T + tb * P : pp * T + (tb + 1) * P],
                            wpj[:, pp * C + oc * 512 : pp * C + (oc + 1) * 512],
                            start=(pp == 0),
                            stop=(pp == 3),
                        )
                    ys = ysp.tile([P, 512], F32, tag="ys")
                    nc.vector.tensor_copy(ys[:], py[:])
                    nc.sync.dma_start(
                        y_d[tb * P : (tb + 1) * P, oc * 512 : (oc + 1) * 512], ys[:]
                    )

    nc.compile()
    return nc


def _shard_inputs(x, W_qkv, b_qkv, W_proj):
    """Build the 8 per-core input maps."""
    in_maps = []
    for c in range(8):
        b = c // 2
        hg = c % 2
        heads = [hg * 8 + j for j in range(8)]
        qk_cols = []
        for p in range(4):
            ha, hb = heads[2 * p], heads[2 * p + 1]
            for part in range(2):  # q, k
                qk_cols.extend(range(ha * 192 + part * 64, ha * 192 + part * 64 + 64))
                qk_cols.extend(range(hb * 192 + part * 64, hb * 192 + part * 64 + 64))
        qk_cols = np.array(qk_cols)
        v_cols = []
        for p in range(4):
            ha, hb = heads[2 * p], heads[2 * p + 1]
            v_cols.extend(range(ha * 192 + 128, ha * 192 + 192))
            v_cols.extend(range(hb * 192 + 128, hb * 192 + 192))
        v_cols = np.array(v_cols)
        in_maps.append(
            {
                "xt": np.ascontiguousarray(x[b].T, dtype=np.float32),
                "wqk": np.ascontiguousarray(W_qkv[:, qk_cols], dtype=np.float32),
                "bqk": np.ascontiguousarray(b_qkv[qk_cols], dtype=np.float32),
                "wv": np.ascontiguousarray(W_qkv[:, v_cols], dtype=np.float32),
                "bv": np.ascontiguousarray(
                    b_qkv[v_cols].reshape(1, 512), dtype=np.float32
                ),
                "wproj": np.ascontiguousarray(
                    W_proj[hg * 512 : (hg + 1) * 512, :], dtype=np.float32
                ),
            }
        )
    return in_maps


_NC = None


def kernel(x, W_qkv, b_qkv, W_proj, b_proj, _trace=False):
    global _NC
    x = np.asarray(x, dtype=np.float32)
    W_qkv = np.asarray(W_qkv, dtype=np.float32)
    b_qkv = np.asarray(b_qkv, dtype=np.float32)
    W_proj = np.asarray(W_proj, dtype=np.float32)
    b_proj = np.asarray(b_proj, dtype=np.float32)

    in_maps = _shard_inputs(x, W_qkv, b_qkv, W_proj)
    if _NC is None:
        _NC = build_kernel()
    res = run_bass_kernel_spmd(
        _NC, in_maps, core_ids=list(range(8)), trace=_trace,
        trace_cores=list(range(8)) if _trace else None,
    )
    out = np.empty((B, T, C), dtype=np.float32)
    for b in range(B):
        out[b] = res.results[2 * b]["y"] + res.results[2 * b + 1]["y"] + b_proj
    if _trace:
        return out, res
    return out


# revision 20
# speedup vs baseline: 1.5494x; 1.1575x over previous
"""Multi-head causal self-attention (B=4, T=2048, C=1024, H=16) on 8 TRN2 cores.

Sharding: core c handles batch b = c//2 and head-group hg = c%2 (8 heads):
data parallel over B, tensor parallel over H. Each core computes qk^T for its
heads (xT @ Wqk column-slice, transposed per-head-pair layout), V in natural
layout, causal attention for its 8 heads, and a partial output projection
(row-split W_proj) -> y_partial [T, C]. Host transposes x per core and sums
y[b] = y_partial[2b] + y_partial[2b+1] + b_proj.

Matmul dtypes: q/k in bf16 (scores accumulate in fp32 PSUM), everything else
float32r (TF32-like). Scores are computed transposed ([k, q]) with
zero-padded q copies so every matmul is full-K at partition base 0. The
softmax denominator arrives via a ones-row folded into the attn@V matmul;
normalization uses a fast approximate reciprocal + K=1 broadcast matmul.
"""

from contextlib import ExitStack

import numpy as np

import concourse.bass as bass
import concourse.bacc as bacc
import concourse.mybir as mybir
import concourse.tile as tile
from concourse.bass_utils import run_bass_kernel_spmd
from concourse.masks import make_upper_triangular

B, T, C, H, HS = 4, 2048, 1024, 16, 64
P = 128
NQC = T // 512          # q-chunks of 512
NKB = T // P            # key blocks of 128
NTB = T // P            # t blocks of 128
TH = T // 2             # t-half
SCALE = HS ** -0.5

F32 = mybir.dt.float32
F32R = mybir.dt.float32r
BF16 = mybir.dt.bfloat16
Exp = mybir.ActivationFunctionType.Exp


def build_kernel():
    nc = bacc.Bacc("TRN2", target_bir_lowering=False)

    xt_d = nc.dram_tensor("xt", (C, T), F32R, kind="ExternalInput")
    wqk_d = nc.dram_tensor("wqk", (C, 8 * P), F32R, kind="ExternalInput")
    bqk_d = nc.dram_tensor("bqk", (8 * P,), F32, kind="ExternalInput")
    wv_d = nc.dram_tensor("wv", (C, 512), F32R, kind="ExternalInput")
    bv_d = nc.dram_tensor("bv", (1, 512), F32R, kind="ExternalInput")
    wproj_d = nc.dram_tensor("wproj", (8 * HS, C), F32R, kind="ExternalInput")
    y_d = nc.dram_tensor("y", (T, C), F32, kind="ExternalOutput")

    with tile.TileContext(nc) as tc, ExitStack() as big:
        const = big.enter_context(tc.tile_pool(name="const", bufs=1))
        persist = big.enter_context(tc.tile_pool(name="persist", bufs=1))

        # mask[k, q] = 1 where k <= q (valid causal entries of a diag block)
        mask = const.tile([P, P], BF16, tag="mask")
        make_upper_triangular(nc, mask[:], val=1.0, diag=True)
        ones_f = const.tile([P, P], F32, tag="ones_f")
        nc.vector.memset(ones_f[:], 1.0)
        ones_t = const.tile([1, P], F32R, tag="ones")
        nc.vector.tensor_copy(ones_t[:], ones_f[0:1, :])

        # qk_all: 12 blocks of [128, T] bf16; per pair p:
        #   block 3p   = qpadA: rows 0:64 q of head 2p, rows 64:128 zero
        #   block 3p+1 = qpadB: rows 0:64 zero, rows 64:128 q of head 2p+1
        #   block 3p+2 = k pair: rows 0:64 k(2p), 64:128 k(2p+1)
        qk_all = persist.tile([P, 12 * T], BF16, tag="qk")
        for p_pair in range(4):
            nc.vector.memset(qk_all[64:P, (3 * p_pair) * T : (3 * p_pair + 1) * T], 0.0)
            nc.vector.memset(qk_all[0:64, (3 * p_pair + 1) * T : (3 * p_pair + 2) * T], 0.0)

        # v_all: per (pair, kb): [vA(64) | onesA(1) | vB(64) | onesB(1)] = 130
        v_all = persist.tile([P, 4 * NKB * 130], BF16, tag="v")
        va4 = v_all[:].rearrange("p (a b c) -> p a b c", a=4, b=NKB, c=130)
        nc.vector.tensor_copy(va4[:, :, :, 64:65], ones_f[:, 0 : 4 * NKB])
        nc.vector.tensor_copy(va4[:, :, :, 129:130], ones_f[:, 0 : 4 * NKB])

        # ---------------- Phase 1: qk^T, natural V ----------------
        with ExitStack() as ph1:
            cp1 = ph1.enter_context(tc.tile_pool(name="cp1", bufs=1))
            xtp = ph1.enter_context(tc.tile_pool(name="xtp", bufs=1))
            wp = ph1.enter_context(tc.tile_pool(name="wp", bufs=2))
            ps_m = ph1.enter_context(tc.tile_pool(name="ps_m", bufs=3, space="PSUM"))
            ps_v = ph1.enter_context(tc.tile_pool(name="ps_v", bufs=3, space="PSUM"))
            ps_c = ph1.enter_context(tc.tile_pool(name="ps_c", bufs=1, space="PSUM"))

            bqk = cp1.tile([P, 8], F32, tag="bqk")
            nc.sync.dma_start(bqk[:], bqk_d[:].rearrange("(a p) -> p a", p=P))
            wv_sb = cp1.tile([P, 8 * 512], F32R, tag="wv")
            nc.sync.dma_start(
                wv_sb[:].rearrange("p (cb j) -> p cb j", cb=8),
                wv_d[:].rearrange("(cb p) j -> p cb j", p=P),
            )
            # bias_v[128, 512] = b_v broadcast along partitions (K=1 matmul)
            bvr = cp1.tile([1, 512], F32R, tag="bvr")
            nc.sync.dma_start(bvr[:], bv_d[:])
            bias_v = cp1.tile([P, 512], F32, tag="bias_v")
            pbv = ps_c.tile([P, 512], F32, tag="pbv")
            nc.tensor.matmul(pbv[:], ones_t[:], bvr[:], start=True, stop=True)
            nc.vector.tensor_copy(bias_v[:], pbv[:])

            for th in range(2):  # t-half
                xT = xtp.tile([P, 8 * TH], F32R, tag="xT")
                for cb in range(8):
                    nc.sync.dma_start(
                        xT[:, cb * TH : (cb + 1) * TH],
                        xt_d[cb * P : (cb + 1) * P, th * TH : (th + 1) * TH],
                    )
                # q, k projections (transposed layout)
                for chb in range(8):
                    p_pair = chb // 2
                    kind = chb % 2  # 0 = q block, 1 = k block
                    wb = wp.tile([P, 8 * P], F32R, tag="w")
                    nc.sync.dma_start(
                        wb[:].rearrange("p (cb j) -> p cb j", cb=8),
                        wqk_d[:, chb * P : (chb + 1) * P].rearrange(
                            "(cb p) j -> p cb j", p=P
                        ),
                    )
                    for tck in range(2):
                        pq = ps_m.tile([P, 512], F32, tag="pq")
                        for cb in range(8):
                            nc.tensor.matmul(
                                pq[:],
                                wb[:, cb * P : (cb + 1) * P],
                                xT[:, cb * TH + tck * 512 : cb * TH + (tck + 1) * 512],
                                start=(cb == 0),
                                stop=(cb == 7),
                            )
                        t0 = th * TH + tck * 512
                        if kind == 0:  # q -> two zero-padded tiles
                            blk_a, blk_b = 3 * p_pair, 3 * p_pair + 1
                            nc.vector.tensor_scalar_add(
                                qk_all[0:64, blk_a * T + t0 : blk_a * T + t0 + 512],
                                pq[0:64, :],
                                bqk[0:64, chb : chb + 1],
                            )
                            nc.vector.tensor_scalar_add(
                                qk_all[64:P, blk_b * T + t0 : blk_b * T + t0 + 512],
                                pq[64:P, :],
                                bqk[64:P, chb : chb + 1],
                            )
                        else:  # k pair block
                            blk = 3 * p_pair + 2
                            nc.vector.tensor_scalar_add(
                                qk_all[:, blk * T + t0 : blk * T + t0 + 512],
                                pq[:],
                                bqk[:, chb : chb + 1],
                            )
                # natural-layout V (xT stationary, wv moving)
                for tb in range(8):
                    kb = th * 8 + tb
                    pv = ps_v.tile([P, 512], F32, tag="pv")
                    for cb in range(8):
                        nc.tensor.matmul(
                            pv[:],
                            xT[:, cb * TH + tb * P : cb * TH + (tb + 1) * P],
                            wv_sb[:, cb * 512 : (cb + 1) * 512],
                            start=(cb == 0),
                            stop=(cb == 7),
                        )
                    dst = bass.AP(
                        v_all[:].tensor,
                        v_all[:].offset + kb * 130,
                        [[v_all[:].ap[0][0], P], [NKB * 130, 4], [65, 2], [1, 64]],
                    )
                    src = bass.AP(
                        pv[:].tensor,
                        pv[:].offset,
                        [[pv[:].ap[0][0], P], [128, 4], [64, 2], [1, 64]],
                    )
                    bsrc = bass.AP(
                        bias_v[:].tensor,
                        bias_v[:].offset,
                        [[bias_v[:].ap[0][0], P], [128, 4], [64, 2], [1, 64]],
                    )
                    nc.vector.tensor_tensor(dst, src, bsrc, mybir.AluOpType.add)

        # aoT: pair-stacked [128 = ch(head 2p) | ch(head 2p+1), 4 * T]
        persist2 = big.enter_context(tc.tile_pool(name="persist2", bufs=1))
        aoT = persist2.tile([P, 4 * T], F32R, tag="aoT")

        # wproj prefetch (DMA overlaps with attention)
        wpp = big.enter_context(tc.tile_pool(name="wpp", bufs=1))
        wpj = wpp.tile([P, 4 * C], F32R, tag="wpj")
        nc.sync.dma_start(
            wpj[:].rearrange("r (pr j) -> r pr j", pr=4),
            wproj_d[:].rearrange("(pr r) j -> r pr j", r=P),
        )

        # ---------------- Phase 2: attention ----------------
        with ExitStack() as ph2:
            atp = ph2.enter_context(tc.tile_pool(name="atp", bufs=2))
            zrp = ph2.enter_context(tc.tile_pool(name="zrp", bufs=2))
            zsp_p = ph2.enter_context(tc.tile_pool(name="zsp_p", bufs=2))
            rzap = ph2.enter_context(tc.tile_pool(name="rzap", bufs=2))
            bcsp = ph2.enter_context(tc.tile_pool(name="bcsp", bufs=3))
            stgp = ph2.enter_context(tc.tile_pool(name="stgp", bufs=3))
            ps_s = ph2.enter_context(tc.tile_pool(name="ps_s", bufs=3, space="PSUM"))
            ps_o = ph2.enter_context(tc.tile_pool(name="ps_o", bufs=2, space="PSUM"))
            ps_b = ph2.enter_context(tc.tile_pool(name="ps_b", bufs=1, space="PSUM"))

            def emit_pair_tail(p_pair, zra, zrb):
                # spread Z rows across 128 partitions, reciprocal, unspread
                zsp = zsp_p.tile([P, 32], F32, tag="zsp")
                for qc in range(NQC):
                    for hh in range(2):
                        r = qc * 2 + hh
                        srcz = (zra if hh == 0 else zrb)[qc * 32 : qc * 32 + 1, :]
                        nc.sync.dma_start(zsp[r * 16 : (r + 1) * 16, :], srcz)
                zspr = zsp_p.tile([P, 32], F32, tag="zspr")
                nc.vector.reciprocal(zspr[:], zsp[:])
                rza = rzap.tile([P, 512], F32, tag="rza")
                rzb = rzap.tile([P, 512], F32, tag="rzb")
                for qc in range(NQC):
                    for hh in range(2):
                        r = qc * 2 + hh
                        dst = (rza if hh == 0 else rzb)[qc * 32 : qc * 32 + 1, :]
                        nc.sync.dma_start(dst, zspr[r * 16 : (r + 1) * 16, :])
                for qc in range(NQC):
                    col = p_pair * T + qc * 512
                    pbt = ps_b.tile([P, 512], F32, tag="pbt")
                    nc.tensor.matmul(
                        pbt[0:64, :],
                        ones_f[qc * 32 : qc * 32 + 1, 0:64],
                        rza[qc * 32 : qc * 32 + 1, :],
                        start=True, stop=True,
                        tile_position=(qc * 32, 0),
                    )
                    nc.tensor.matmul(
                        pbt[64:P, :],
                        ones_f[qc * 32 : qc * 32 + 1, 0:64],
                        rzb[qc * 32 : qc * 32 + 1, :],
                        start=True, stop=True,
                        skip_group_check=True,
                        tile_position=(qc * 32, 64),
                    )
                    bcs = bcsp.tile([P, 512], F32, tag="bcs")
                    nc.vector.tensor_copy(bcs[:], pbt[:])
                    nc.vector.tensor_mul(
                        aoT[0:64, col : col + 512],
                        aoT[0:64, col : col + 512],
                        bcs[0:64, :],
                    )
                    nc.vector.tensor_mul(
                        aoT[64:P, col : col + 512],
                        aoT[64:P, col : col + 512],
                        bcs[64:P, :],
                    )

            pending = []
            for p_pair in range(4):
                kblk = 3 * p_pair + 2
                zra = zrp.tile([P, 512], F32, tag="zra")
                zrb = zrp.tile([P, 512], F32, tag="zrb")
                for qc in range(NQC):
                    po0 = ps_o.tile([65, 512], F32, tag="po0")
                    po1 = ps_o.tile([65, 512], F32, tag="po1")
                    po = [po0, po1]
                    nkb = 4 * qc + 4
                    for kb in range(nkb):
                        qoff = max(0, kb * P - qc * 512)
                        for hh in range(2):
                            qblk = 3 * p_pair + hh
                            ps = ps_s.tile([P, 512], F32, tag="ps")
                            nc.tensor.matmul(
                                ps[:, qoff:512],
                                qk_all[:, kblk * T + kb * P : kblk * T + (kb + 1) * P],
                                qk_all[:, qblk * T + qc * 512 + qoff : qblk * T + (qc + 1) * 512],
                                start=True,
                                stop=True,
                            )
                            at = atp.tile([P, 512], BF16, tag=f"at{hh}")
                            nc.scalar.activation(
                                at[:, qoff:512], ps[:, qoff:512], Exp, scale=SCALE,
                            )
                            if kb * P >= qc * 512:
                                # diagonal block: zero out k > q entries
                                nc.vector.tensor_mul(
                                    at[:, qoff : qoff + P],
                                    at[:, qoff : qoff + P],
                                    mask[:],
                                )
                            nc.tensor.matmul(
                                po[hh][:, qoff:512],
                                v_all[:, p_pair * NKB * 130 + kb * 130 + hh * 65 :
                                      p_pair * NKB * 130 + kb * 130 + hh * 65 + 65],
                                at[:, qoff:512],
                                start=(kb == 0),
                                stop=(kb == nkb - 1),
                                skip_group_check=True,
                            )
                    # evict raw ao + Z rows; normalization deferred one pair
                    col = p_pair * T + qc * 512
                    nc.vector.tensor_copy(
                        aoT[0:64, col : col + 512], po0[0:64, :]
                    )
                    stg = stgp.tile([64, 512], F32R, tag="stg")
                    nc.vector.tensor_copy(stg[:], po1[0:64, :])
                    nc.sync.dma_start(aoT[64:P, col : col + 512], stg[:])
                    nc.vector.tensor_copy(zra[qc * 32 : qc * 32 + 1, :], po0[64:65, :])
                    nc.vector.tensor_copy(zrb[qc * 32 : qc * 32 + 1, :], po1[64:65, :])
                pending.append((p_pair, zra, zrb))
                if len(pending) >= 2:
                    emit_pair_tail(*pending.pop(0))
            for args in pending:
                emit_pair_tail(*args)

        # ---------------- Phase 3: output projection (partial) ----------------
        with ExitStack() as ph3:
            ysp = ph3.enter_context(tc.tile_pool(name="ysp", bufs=3))
            ps_y = ph3.enter_context(tc.tile_pool(name="ps_y", bufs=3, space="PSUM"))

            for tb in range(NTB):
                for oc in range(2):
                    py = ps_y.tile([P, 512], F32, tag="py")
                    for pp in range(4):
                        nc.tensor.matmul(
                            py[:],
                            aoT[:, pp * T + tb * P : pp * T + (tb + 1) * P],
                            wpj[:, pp * C + oc * 512 : pp * C + (oc + 1) * 512],
                            start=(pp == 0),
                            stop=(pp == 3),
                        )
                    ys = ysp.tile([P, 512], F32, tag="ys")
                    nc.vector.tensor_copy(ys[:], py[:])
                    nc.sync.dma_start(
                        y_d[tb * P : (tb + 1) * P, oc * 512 : (oc + 1) * 512], ys[:]
                    )

    nc.compile()
    return nc


def _shard_inputs(x, W_qkv, b_qkv, W_proj):
    """Build the 8 per-core input maps."""
    in_maps = []
    for c in range(8):
        b = c // 2
        hg = c % 2
        heads = [hg * 8 + j for j in range(8)]
        qk_cols = []
        for p in range(4):
            ha, hb = heads[2 * p], heads[2 * p + 1]
            for part in range(2):  # q, k
                qk_cols.extend(range(ha * 192 + part * 64, ha * 192 + part * 64 + 64))
                qk_cols.extend(range(hb * 192 + part * 64, hb * 192 + part * 64 + 64))
        qk_cols = np.array(qk_cols)
        v_cols = []
        for p in range(4):
            ha, hb = heads[2 * p], heads[2 * p + 1]
            v_cols.extend(range(ha * 192 + 128, ha * 192 + 192))
            v_cols.extend(range(hb * 192 + 128, hb * 192 + 192))
        v_cols = np.array(v_cols)
        in_maps.append(
            {
                "xt": np.ascontiguousarray(x[b].T, dtype=np.float32),
                "wqk": np.ascontiguousarray(W_qkv[:, qk_cols], dtype=np.float32),
                "bqk": np.ascontiguousarray(b_qkv[qk_cols], dtype=np.float32),
                "wv": np.ascontiguousarray(W_qkv[:, v_cols], dtype=np.float32),
                "bv": np.ascontiguousarray(
                    b_qkv[v_cols].reshape(1, 512), dtype=np.float32
                ),
                "wproj": np.ascontiguousarray(
                    W_proj[hg * 512 : (hg + 1) * 512, :], dtype=np.float32
                ),
            }
        )
    return in_maps


_NC = None


def kernel(x, W_qkv, b_qkv, W_proj, b_proj, _trace=False):
    global _NC
    x = np.asarray(x, dtype=np.float32)
    W_qkv = np.asarray(W_qkv, dtype=np.float32)
    b_qkv = np.asarray(b_qkv, dtype=np.float32)
    W_proj = np.asarray(W_proj, dtype=np.float32)
    b_proj = np.asarray(b_proj, dtype=np.float32)

    in_maps = _shard_inputs(x, W_qkv, b_qkv, W_proj)
    if _NC is None:
        _NC = build_kernel()
    res = run_bass_kernel_spmd(
        _NC, in_maps, core_ids=list(range(8)), trace=_trace,
        trace_cores=list(range(8)) if _trace else None,
    )
    out = np.empty((B, T, C), dtype=np.float32)
    for b in range(B):
        out[b] = res.results[2 * b]["y"] + res.results[2 * b + 1]["y"] + b_proj
    if _trace:
        return out, res
    return out


# revision 21
# speedup vs baseline: 1.6553x; 1.0683x over previous
"""Multi-head causal self-attention (B=4, T=2048, C=1024, H=16) on 8 TRN2 cores.

Sharding: core c handles batch b = c//2 and head-group hg = c%2 (8 heads):
data parallel over B, tensor parallel over H. Each core computes qk^T for its
heads (xT @ Wqk column-slice, transposed per-head-pair layout), V in natural
layout, causal attention for its 8 heads, and a partial output projection
(row-split W_proj) -> y_partial [T, C]. Host transposes x per core and sums
y[b] = y_partial[2b] + y_partial[2b+1] + b_proj.

Matmul dtypes: q/k in bf16 (scores accumulate in fp32 PSUM), everything else
float32r (TF32-like). Scores are computed transposed ([k, q]) with
zero-padded q copies so every matmul is full-K at partition base 0. The
softmax denominator arrives via a ones-row folded into the attn@V matmul;
normalization uses a fast approximate reciprocal + K=1 broadcast matmul.
"""

from contextlib import ExitStack

import numpy as np

import concourse.bass as bass
import concourse.bacc as bacc
import concourse.mybir as mybir
import concourse.tile as tile
from concourse.bass_utils import run_bass_kernel_spmd
from concourse.masks import make_upper_triangular

B, T, C, H, HS = 4, 2048, 1024, 16, 64
P = 128
NQC = T // 512          # q-chunks of 512
NKB = T // P            # key blocks of 128
NTB = T // P            # t blocks of 128
TH = T // 2             # t-half
SCALE = HS ** -0.5

F32 = mybir.dt.float32
F32R = mybir.dt.float32r
BF16 = mybir.dt.bfloat16
Exp = mybir.ActivationFunctionType.Exp


def build_kernel():
    nc = bacc.Bacc("TRN2", target_bir_lowering=False)

    xt_d = nc.dram_tensor("xt", (C, T), F32R, kind="ExternalInput")
    wqk_d = nc.dram_tensor("wqk", (C, 8 * P), F32R, kind="ExternalInput")
    bqk_d = nc.dram_tensor("bqk", (8 * P,), F32, kind="ExternalInput")
    wv_d = nc.dram_tensor("wv", (C, 512), F32R, kind="ExternalInput")
    bv_d = nc.dram_tensor("bv", (1, 512), F32R, kind="ExternalInput")
    wproj_d = nc.dram_tensor("wproj", (8 * HS, C), F32R, kind="ExternalInput")
    y_d = nc.dram_tensor("y", (T, C), F32, kind="ExternalOutput")

    with tile.TileContext(nc) as tc, ExitStack() as big:
        const = big.enter_context(tc.tile_pool(name="const", bufs=1))
        persist = big.enter_context(tc.tile_pool(name="persist", bufs=1))

        # mask[k, q] = 1 where k <= q (valid causal entries of a diag block)
        mask = const.tile([P, P], BF16, tag="mask")
        make_upper_triangular(nc, mask[:], val=1.0, diag=True)
        ones_f = const.tile([P, P], F32, tag="ones_f")
        nc.vector.memset(ones_f[:], 1.0)
        ones_t = const.tile([1, P], F32R, tag="ones")
        nc.vector.tensor_copy(ones_t[:], ones_f[0:1, :])

        # qk_all: 12 blocks of [128, T] bf16; per pair p:
        #   block 3p   = qpadA: rows 0:64 q of head 2p, rows 64:128 zero
        #   block 3p+1 = qpadB: rows 0:64 zero, rows 64:128 q of head 2p+1
        #   block 3p+2 = k pair: rows 0:64 k(2p), 64:128 k(2p+1)
        qk_all = persist.tile([P, 12 * T], BF16, tag="qk")
        for p_pair in range(4):
            nc.vector.memset(qk_all[64:P, (3 * p_pair) * T : (3 * p_pair + 1) * T], 0.0)
            nc.vector.memset(qk_all[0:64, (3 * p_pair + 1) * T : (3 * p_pair + 2) * T], 0.0)

        # v_all: per (pair, kb): [vA(64) | onesA(1) | vB(64) | onesB(1)] = 130
        v_all = persist.tile([P, 4 * NKB * 130], BF16, tag="v")
        va4 = v_all[:].rearrange("p (a b c) -> p a b c", a=4, b=NKB, c=130)
        nc.vector.tensor_copy(va4[:, :, :, 64:65], ones_f[:, 0 : 4 * NKB])
        nc.vector.tensor_copy(va4[:, :, :, 129:130], ones_f[:, 0 : 4 * NKB])

        # ---------------- Phase 1: qk^T, natural V ----------------
        with ExitStack() as ph1:
            cp1 = ph1.enter_context(tc.tile_pool(name="cp1", bufs=1))
            xtp = ph1.enter_context(tc.tile_pool(name="xtp", bufs=1))
            wp = ph1.enter_context(tc.tile_pool(name="wp", bufs=2))
            ps_m = ph1.enter_context(tc.tile_pool(name="ps_m", bufs=3, space="PSUM"))
            ps_v = ph1.enter_context(tc.tile_pool(name="ps_v", bufs=3, space="PSUM"))
            ps_c = ph1.enter_context(tc.tile_pool(name="ps_c", bufs=1, space="PSUM"))

            bqk = cp1.tile([P, 8], F32, tag="bqk")
            nc.sync.dma_start(bqk[:], bqk_d[:].rearrange("(a p) -> p a", p=P))
            wv_sb = cp1.tile([P, 8 * 512], F32R, tag="wv")
            nc.sync.dma_start(
                wv_sb[:].rearrange("p (cb j) -> p cb j", cb=8),
                wv_d[:].rearrange("(cb p) j -> p cb j", p=P),
            )
            # bias_v[128, 512] = b_v broadcast along partitions (K=1 matmul)
            bvr = cp1.tile([1, 512], F32R, tag="bvr")
            nc.sync.dma_start(bvr[:], bv_d[:])
            bias_v = cp1.tile([P, 512], F32, tag="bias_v")
            pbv = ps_c.tile([P, 512], F32, tag="pbv")
            nc.tensor.matmul(pbv[:], ones_t[:], bvr[:], start=True, stop=True)
            nc.vector.tensor_copy(bias_v[:], pbv[:])

            for th in range(2):  # t-half
                xT = xtp.tile([P, 8 * TH], F32R, tag="xT")
                for cb in range(8):
                    nc.sync.dma_start(
                        xT[:, cb * TH : (cb + 1) * TH],
                        xt_d[cb * P : (cb + 1) * P, th * TH : (th + 1) * TH],
                    )
                # q, k projections (transposed layout)
                for chb in range(8):
                    p_pair = chb // 2
                    kind = chb % 2  # 0 = q block, 1 = k block
                    wb = wp.tile([P, 8 * P], F32R, tag="w")
                    nc.sync.dma_start(
                        wb[:].rearrange("p (cb j) -> p cb j", cb=8),
                        wqk_d[:, chb * P : (chb + 1) * P].rearrange(
                            "(cb p) j -> p cb j", p=P
                        ),
                    )
                    for tck in range(2):
                        pq = ps_m.tile([P, 512], F32, tag="pq")
                        for cb in range(8):
                            nc.tensor.matmul(
                                pq[:],
                                wb[:, cb * P : (cb + 1) * P],
                                xT[:, cb * TH + tck * 512 : cb * TH + (tck + 1) * 512],
                                start=(cb == 0),
                                stop=(cb == 7),
                            )
                        t0 = th * TH + tck * 512
                        if kind == 0:  # q -> two zero-padded tiles
                            blk_a, blk_b = 3 * p_pair, 3 * p_pair + 1
                            nc.vector.tensor_scalar_add(
                                qk_all[0:64, blk_a * T + t0 : blk_a * T + t0 + 512],
                                pq[0:64, :],
                                bqk[0:64, chb : chb + 1],
                            )
                            nc.vector.tensor_scalar_add(
                                qk_all[64:P, blk_b * T + t0 : blk_b * T + t0 + 512],
                                pq[64:P, :],
                                bqk[64:P, chb : chb + 1],
                            )
                        else:  # k pair block
                            blk = 3 * p_pair + 2
                            nc.vector.tensor_scalar_add(
                                qk_all[:, blk * T + t0 : blk * T + t0 + 512],
                                pq[:],
                                bqk[:, chb : chb + 1],
                            )
                # natural-layout V (xT stationary, wv moving)
                for tb in range(8):
                    kb = th * 8 + tb
                    pv = ps_v.tile([P, 512], F32, tag="pv")
                    for cb in range(8):
                        nc.tensor.matmul(
                            pv[:],
                            xT[:, cb * TH + tb * P : cb * TH + (tb + 1) * P],
                            wv_sb[:, cb * 512 : (cb + 1) * 512],
                            start=(cb == 0),
                            stop=(cb == 7),
                        )
                    dst = bass.AP(
                        v_all[:].tensor,
                        v_all[:].offset + kb * 130,
                        [[v_all[:].ap[0][0], P], [NKB * 130, 4], [65, 2], [1, 64]],
                    )
                    src = bass.AP(
                        pv[:].tensor,
                        pv[:].offset,
                        [[pv[:].ap[0][0], P], [128, 4], [64, 2], [1, 64]],
                    )
                    bsrc = bass.AP(
                        bias_v[:].tensor,
                        bias_v[:].offset,
                        [[bias_v[:].ap[0][0], P], [128, 4], [64, 2], [1, 64]],
                    )
                    nc.vector.tensor_tensor(dst, src, bsrc, mybir.AluOpType.add)

        # aoT: pair-stacked [128 = ch(head 2p) | ch(head 2p+1), 4 * T]
        persist2 = big.enter_context(tc.tile_pool(name="persist2", bufs=1))
        aoT = persist2.tile([P, 4 * T], F32R, tag="aoT")

        # sel2: rows {32p: cols 0:64 = 1}, {32p+1: cols 64:128 = 1}, else 0
        sel2 = const.tile([P, P], F32, tag="sel2")
        nc.vector.memset(sel2[:], 0.0)
        for pr in range(4):
            nc.sync.dma_start(sel2[pr * 32 : pr * 32 + 1, 0:64], ones_f[0:1, 0:64])
            nc.sync.dma_start(
                sel2[pr * 32 + 1 : pr * 32 + 2, 64:P], ones_f[0:1, 0:64]
            )

        # wproj prefetch (DMA overlaps with attention)
        wpp = big.enter_context(tc.tile_pool(name="wpp", bufs=1))
        wpj = wpp.tile([P, 4 * C], F32R, tag="wpj")
        nc.sync.dma_start(
            wpj[:].rearrange("r (pr j) -> r pr j", pr=4),
            wproj_d[:].rearrange("(pr r) j -> r pr j", r=P),
        )

        # ---------------- Phase 2+3: attention + interleaved projection ----
        with ExitStack() as ph2:
            atp = ph2.enter_context(tc.tile_pool(name="atp", bufs=2))
            zrp = ph2.enter_context(tc.tile_pool(name="zrp", bufs=2))
            zsp_p = ph2.enter_context(tc.tile_pool(name="zsp_p", bufs=2))
            rzap = ph2.enter_context(tc.tile_pool(name="rzap", bufs=2))
            bcsp = ph2.enter_context(tc.tile_pool(name="bcsp", bufs=3))
            stgp = ph2.enter_context(tc.tile_pool(name="stgp", bufs=3))
            ysp = ph2.enter_context(tc.tile_pool(name="ysp", bufs=3))
            ps_s = ph2.enter_context(tc.tile_pool(name="ps_s", bufs=3, space="PSUM"))
            ps_o = ph2.enter_context(tc.tile_pool(name="ps_o", bufs=1, space="PSUM"))
            ps_b = ph2.enter_context(tc.tile_pool(name="ps_b", bufs=1, space="PSUM"))
            ps_y = ph2.enter_context(tc.tile_pool(name="ps_y", bufs=2, space="PSUM"))

            def emit_round_tail(qc, zra, zrb):
                # spread Z rows across 128 partitions, reciprocal, unspread
                zsp = zsp_p.tile([P, 32], F32, tag="zsp")
                for pr in range(4):
                    for hh in range(2):
                        r = pr * 2 + hh
                        srcz = (zra if hh == 0 else zrb)[pr * 32 : pr * 32 + 1, :]
                        nc.sync.dma_start(zsp[r * 16 : (r + 1) * 16, :], srcz)
                zspr = zsp_p.tile([P, 32], F32, tag="zspr")
                nc.vector.reciprocal(zspr[:], zsp[:])
                rz2 = rzap.tile([P, 512], F32, tag="rz2")
                for pr in range(4):
                    for hh in range(2):
                        r = pr * 2 + hh
                        nc.sync.dma_start(
                            rz2[pr * 32 + hh : pr * 32 + hh + 1, :],
                            zspr[r * 16 : (r + 1) * 16, :],
                        )
                for pr in range(4):
                    col = pr * T + qc * 512
                    pbt = ps_b.tile([P, 512], F32, tag="pbt")
                    nc.tensor.matmul(
                        pbt[:],
                        sel2[pr * 32 : pr * 32 + 2, :],
                        rz2[pr * 32 : pr * 32 + 2, :],
                        start=True, stop=True,
                        tile_position=(pr * 32, 0),
                    )
                    bcs = bcsp.tile([P, 512], F32, tag="bcs")
                    nc.vector.tensor_copy(bcs[:], pbt[:])
                    nc.vector.tensor_mul(
                        aoT[0:64, col : col + 512],
                        aoT[0:64, col : col + 512],
                        bcs[0:64, :],
                    )
                    nc.vector.tensor_mul(
                        aoT[64:P, col : col + 512],
                        aoT[64:P, col : col + 512],
                        bcs[64:P, :],
                    )

            def emit_proj_round(qc):
                for tb in range(qc * 4, (qc + 1) * 4):
                    for oc in range(2):
                        py = ps_y.tile([P, 512], F32, tag="py")
                        for pp in range(4):
                            nc.tensor.matmul(
                                py[:],
                                aoT[:, pp * T + tb * P : pp * T + (tb + 1) * P],
                                wpj[:, pp * C + oc * 512 : pp * C + (oc + 1) * 512],
                                start=(pp == 0),
                                stop=(pp == 3),
                            )
                        ys = ysp.tile([P, 512], F32, tag="ys")
                        nc.vector.tensor_copy(ys[:], py[:])
                        nc.sync.dma_start(
                            y_d[tb * P : (tb + 1) * P, oc * 512 : (oc + 1) * 512],
                            ys[:],
                        )

            pending = None
            for qc in range(NQC):
                zra = zrp.tile([P, 512], F32, tag="zra")
                zrb = zrp.tile([P, 512], F32, tag="zrb")
                for p_pair in range(4):
                    kblk = 3 * p_pair + 2
                    po0 = ps_o.tile([65, 512], F32, tag="po0")
                    po1 = ps_o.tile([65, 512], F32, tag="po1")
                    po = [po0, po1]
                    nkb = 4 * qc + 4
                    for kb in range(nkb):
                        qoff = max(0, kb * P - qc * 512)
                        for hh in range(2):
                            qblk = 3 * p_pair + hh
                            ps = ps_s.tile([P, 512], F32, tag="ps")
                            nc.tensor.matmul(
                                ps[:, qoff:512],
                                qk_all[:, kblk * T + kb * P : kblk * T + (kb + 1) * P],
                                qk_all[:, qblk * T + qc * 512 + qoff : qblk * T + (qc + 1) * 512],
                                start=True,
                                stop=True,
                            )
                            at = atp.tile([P, 512], BF16, tag=f"at{hh}")
                            nc.scalar.activation(
                                at[:, qoff:512], ps[:, qoff:512], Exp, scale=SCALE,
                            )
                            if kb * P >= qc * 512:
                                # diagonal block: zero out k > q entries
                                nc.vector.tensor_mul(
                                    at[:, qoff : qoff + P],
                                    at[:, qoff : qoff + P],
                                    mask[:],
                                )
                            nc.tensor.matmul(
                                po[hh][:, qoff:512],
                                v_all[:, p_pair * NKB * 130 + kb * 130 + hh * 65 :
                                      p_pair * NKB * 130 + kb * 130 + hh * 65 + 65],
                                at[:, qoff:512],
                                start=(kb == 0),
                                stop=(kb == nkb - 1),
                                skip_group_check=True,
                            )
                    # evict raw ao + Z rows; normalization deferred one round
                    col = p_pair * T + qc * 512
                    nc.vector.tensor_copy(aoT[0:64, col : col + 512], po0[0:64, :])
                    stg = stgp.tile([64, 512], F32R, tag="stg")
                    nc.vector.tensor_copy(stg[:], po1[0:64, :])
                    nc.sync.dma_start(aoT[64:P, col : col + 512], stg[:])
                    nc.vector.tensor_copy(
                        zra[p_pair * 32 : p_pair * 32 + 1, :], po0[64:65, :]
                    )
                    nc.vector.tensor_copy(
                        zrb[p_pair * 32 : p_pair * 32 + 1, :], po1[64:65, :]
                    )
                if pending is not None:
                    emit_round_tail(*pending)
                    emit_proj_round(pending[0])
                pending = (qc, zra, zrb)
            emit_round_tail(*pending)
            emit_proj_round(pending[0])

    nc.compile()
    return nc


def _shard_inputs(x, W_qkv, b_qkv, W_proj):
    """Build the 8 per-core input maps."""
    in_maps = []
    for c in range(8):
        b = c // 2
        hg = c % 2
        heads = [hg * 8 + j for j in range(8)]
        qk_cols = []
        for p in range(4):
            ha, hb = heads[2 * p], heads[2 * p + 1]
            for part in range(2):  # q, k
                qk_cols.extend(range(ha * 192 + part * 64, ha * 192 + part * 64 + 64))
                qk_cols.extend(range(hb * 192 + part * 64, hb * 192 + part * 64 + 64))
        qk_cols = np.array(qk_cols)
        v_cols = []
        for p in range(4):
            ha, hb = heads[2 * p], heads[2 * p + 1]
            v_cols.extend(range(ha * 192 + 128, ha * 192 + 192))
            v_cols.extend(range(hb * 192 + 128, hb * 192 + 192))
        v_cols = np.array(v_cols)
        in_maps.append(
            {
                "xt": np.ascontiguousarray(x[b].T, dtype=np.float32),
                "wqk": np.ascontiguousarray(W_qkv[:, qk_cols], dtype=np.float32),
                "bqk": np.ascontiguousarray(b_qkv[qk_cols], dtype=np.float32),
                "wv": np.ascontiguousarray(W_qkv[:, v_cols], dtype=np.float32),
                "bv": np.ascontiguousarray(
                    b_qkv[v_cols].reshape(1, 512), dtype=np.float32
                ),
                "wproj": np.ascontiguousarray(
                    W_proj[hg * 512 : (hg + 1) * 512, :], dtype=np.float32
                ),
            }
        )
    return in_maps


_NC = None


def kernel(x, W_qkv, b_qkv, W_proj, b_proj, _trace=False):
    global _NC
    x = np.asarray(x, dtype=np.float32)
    W_qkv = np.asarray(W_qkv, dtype=np.float32)
    b_qkv = np.asarray(b_qkv, dtype=np.float32)
    W_proj = np.asarray(W_proj, dtype=np.float32)
    b_proj = np.asarray(b_proj, dtype=np.float32)

    in_maps = _shard_inputs(x, W_qkv, b_qkv, W_proj)
    if _NC is None:
        _NC = build_kernel()
    res = run_bass_kernel_spmd(
        _NC, in_maps, core_ids=list(range(8)), trace=_trace,
        trace_cores=list(range(8)) if _trace else None,
    )
    out = np.empty((B, T, C), dtype=np.float32)
    for b in range(B):
        out[b] = res.results[2 * b]["y"] + res.results[2 * b + 1]["y"] + b_proj
    if _trace:
        return out, res
    return out


# revision 22
# speedup vs baseline: 1.7291x; 1.0446x over previous
"""Multi-head causal self-attention (B=4, T=2048, C=1024, H=16) on 8 TRN2 cores.

Sharding: core c handles batch b = c//2 and head-group hg = c%2 (8 heads):
data parallel over B, tensor parallel over H. Each core computes qk^T for its
heads (xT @ Wqk column-slice, transposed per-head-pair layout), V in natural
layout, causal attention for its 8 heads, and a partial output projection
(row-split W_proj) -> y_partial [T, C]. Host transposes x per core and sums
y[b] = y_partial[2b] + y_partial[2b+1] + b_proj.

Matmul dtypes: q/k in bf16 (scores accumulate in fp32 PSUM), everything else
float32r (TF32-like). Scores are computed transposed ([k, q]) with
zero-padded q copies so every matmul is full-K at partition base 0. The
softmax denominator arrives via a ones-row folded into the attn@V matmul;
normalization uses a fast approximate reciprocal + K=1 broadcast matmul.
"""

from contextlib import ExitStack

import numpy as np

import concourse.bass as bass
import concourse.bacc as bacc
import concourse.mybir as mybir
import concourse.tile as tile
from concourse.bass_utils import run_bass_kernel_spmd
from concourse.masks import make_upper_triangular

B, T, C, H, HS = 4, 2048, 1024, 16, 64
P = 128
NQC = T // 512          # q-chunks of 512
NKB = T // P            # key blocks of 128
NTB = T // P            # t blocks of 128
TH = T // 2             # t-half
SCALE = HS ** -0.5

F32 = mybir.dt.float32
F32R = mybir.dt.float32r
BF16 = mybir.dt.bfloat16
Exp = mybir.ActivationFunctionType.Exp


def build_kernel():
    nc = bacc.Bacc("TRN2", target_bir_lowering=False)

    xt_d = nc.dram_tensor("xt", (C, T), F32R, kind="ExternalInput")
    wqk_d = nc.dram_tensor("wqk", (C, 8 * P), F32R, kind="ExternalInput")
    bqk_d = nc.dram_tensor("bqk", (8 * P,), F32, kind="ExternalInput")
    wv_d = nc.dram_tensor("wv", (C, 512), F32R, kind="ExternalInput")
    bv_d = nc.dram_tensor("bv", (1, 512), F32R, kind="ExternalInput")
    wproj_d = nc.dram_tensor("wproj", (8 * HS, C), F32R, kind="ExternalInput")
    y_d = nc.dram_tensor("y", (T, C), F32, kind="ExternalOutput")

    with tile.TileContext(nc) as tc, ExitStack() as big:
        const = big.enter_context(tc.tile_pool(name="const", bufs=1))
        persist = big.enter_context(tc.tile_pool(name="persist", bufs=1))

        # mask[k, q] = 1 where k <= q (valid causal entries of a diag block)
        mask = const.tile([P, P], BF16, tag="mask")
        make_upper_triangular(nc, mask[:], val=1.0, diag=True)
        ones_f = const.tile([P, P], F32, tag="ones_f")
        nc.vector.memset(ones_f[:], 1.0)
        ones_t = const.tile([1, P], F32R, tag="ones")
        nc.vector.tensor_copy(ones_t[:], ones_f[0:1, :])

        # qk_all: 12 blocks of [128, T] bf16; per pair p:
        #   block 3p   = qpadA: rows 0:64 q of head 2p, rows 64:128 zero
        #   block 3p+1 = qpadB: rows 0:64 zero, rows 64:128 q of head 2p+1
        #   block 3p+2 = k pair: rows 0:64 k(2p), 64:128 k(2p+1)
        qk_all = persist.tile([P, 12 * T], BF16, tag="qk")
        for p_pair in range(4):
            nc.vector.memset(qk_all[64:P, (3 * p_pair) * T : (3 * p_pair + 1) * T], 0.0)
            nc.vector.memset(qk_all[0:64, (3 * p_pair + 1) * T : (3 * p_pair + 2) * T], 0.0)

        # v_all: per (pair, kb): [vA(64) | onesA(1) | vB(64) | onesB(1)] = 130
        v_all = persist.tile([P, 4 * NKB * 130], BF16, tag="v")
        va4 = v_all[:].rearrange("p (a b c) -> p a b c", a=4, b=NKB, c=130)
        nc.vector.tensor_copy(va4[:, :, :, 64:65], ones_f[:, 0 : 4 * NKB])
        nc.vector.tensor_copy(va4[:, :, :, 129:130], ones_f[:, 0 : 4 * NKB])

        # ---------------- Phase 1: qk^T, natural V ----------------
        with ExitStack() as ph1:
            cp1 = ph1.enter_context(tc.tile_pool(name="cp1", bufs=1))
            xtp = ph1.enter_context(tc.tile_pool(name="xtp", bufs=2))
            wp = ph1.enter_context(tc.tile_pool(name="wp", bufs=2))
            ps_m = ph1.enter_context(tc.tile_pool(name="ps_m", bufs=3, space="PSUM"))
            ps_v = ph1.enter_context(tc.tile_pool(name="ps_v", bufs=3, space="PSUM"))
            ps_c = ph1.enter_context(tc.tile_pool(name="ps_c", bufs=1, space="PSUM"))

            bqk = cp1.tile([P, 8], F32, tag="bqk")
            nc.sync.dma_start(bqk[:], bqk_d[:].rearrange("(a p) -> p a", p=P))
            wv_sb = cp1.tile([P, 8 * 512], F32R, tag="wv")
            nc.sync.dma_start(
                wv_sb[:].rearrange("p (cb j) -> p cb j", cb=8),
                wv_d[:].rearrange("(cb p) j -> p cb j", p=P),
            )
            # bias_v[128, 512] = b_v broadcast along partitions (K=1 matmul)
            bvr = cp1.tile([1, 512], F32R, tag="bvr")
            nc.sync.dma_start(bvr[:], bv_d[:])
            bias_v = cp1.tile([P, 512], F32, tag="bias_v")
            pbv = ps_c.tile([P, 512], F32, tag="pbv")
            nc.tensor.matmul(pbv[:], ones_t[:], bvr[:], start=True, stop=True)
            nc.vector.tensor_copy(bias_v[:], pbv[:])

            for th in range(2):  # t-half
                xT = xtp.tile([P, 8 * TH], F32R, tag="xT")
                for cb in range(8):
                    nc.sync.dma_start(
                        xT[:, cb * TH : (cb + 1) * TH],
                        xt_d[cb * P : (cb + 1) * P, th * TH : (th + 1) * TH],
                    )
                # q, k projections (transposed layout)
                for chb in range(8):
                    p_pair = chb // 2
                    kind = chb % 2  # 0 = q block, 1 = k block
                    wb = wp.tile([P, 8 * P], F32R, tag="w")
                    nc.sync.dma_start(
                        wb[:].rearrange("p (cb j) -> p cb j", cb=8),
                        wqk_d[:, chb * P : (chb + 1) * P].rearrange(
                            "(cb p) j -> p cb j", p=P
                        ),
                    )
                    for tck in range(2):
                        pq = ps_m.tile([P, 512], F32, tag="pq")
                        for cb in range(8):
                            nc.tensor.matmul(
                                pq[:],
                                wb[:, cb * P : (cb + 1) * P],
                                xT[:, cb * TH + tck * 512 : cb * TH + (tck + 1) * 512],
                                start=(cb == 0),
                                stop=(cb == 7),
                            )
                        t0 = th * TH + tck * 512
                        if kind == 0:  # q -> two zero-padded tiles
                            blk_a, blk_b = 3 * p_pair, 3 * p_pair + 1
                            nc.vector.tensor_scalar_add(
                                qk_all[0:64, blk_a * T + t0 : blk_a * T + t0 + 512],
                                pq[0:64, :],
                                bqk[0:64, chb : chb + 1],
                            )
                            nc.vector.tensor_scalar_add(
                                qk_all[64:P, blk_b * T + t0 : blk_b * T + t0 + 512],
                                pq[64:P, :],
                                bqk[64:P, chb : chb + 1],
                            )
                        else:  # k pair block
                            blk = 3 * p_pair + 2
                            nc.vector.tensor_scalar_add(
                                qk_all[:, blk * T + t0 : blk * T + t0 + 512],
                                pq[:],
                                bqk[:, chb : chb + 1],
                            )
                # natural-layout V (xT stationary, wv moving)
                for tb in range(8):
                    kb = th * 8 + tb
                    pv = ps_v.tile([P, 512], F32, tag="pv")
                    for cb in range(8):
                        nc.tensor.matmul(
                            pv[:],
                            xT[:, cb * TH + tb * P : cb * TH + (tb + 1) * P],
                            wv_sb[:, cb * 512 : (cb + 1) * 512],
                            start=(cb == 0),
                            stop=(cb == 7),
                        )
                    dst = bass.AP(
                        v_all[:].tensor,
                        v_all[:].offset + kb * 130,
                        [[v_all[:].ap[0][0], P], [NKB * 130, 4], [65, 2], [1, 64]],
                    )
                    src = bass.AP(
                        pv[:].tensor,
                        pv[:].offset,
                        [[pv[:].ap[0][0], P], [128, 4], [64, 2], [1, 64]],
                    )
                    bsrc = bass.AP(
                        bias_v[:].tensor,
                        bias_v[:].offset,
                        [[bias_v[:].ap[0][0], P], [128, 4], [64, 2], [1, 64]],
                    )
                    nc.vector.tensor_tensor(dst, src, bsrc, mybir.AluOpType.add)

        # aoT: pair-stacked [128 = ch(head 2p) | ch(head 2p+1), 4 * T]
        persist2 = big.enter_context(tc.tile_pool(name="persist2", bufs=1))
        aoT = persist2.tile([P, 4 * T], F32R, tag="aoT")

        # sel2: rows {32p: cols 0:64 = 1}, {32p+1: cols 64:128 = 1}, else 0
        sel2 = const.tile([P, P], F32R, tag="sel2")
        nc.vector.memset(sel2[:].bitcast(F32), 0.0)
        for pr in range(4):
            nc.sync.dma_start(sel2[pr * 32 : pr * 32 + 1, 0:64].bitcast(F32), ones_f[0:1, 0:64])
            nc.sync.dma_start(
                sel2[pr * 32 + 1 : pr * 32 + 2, 64:P].bitcast(F32), ones_f[0:1, 0:64]
            )

        # wproj prefetch (DMA overlaps with attention)
        wpp = big.enter_context(tc.tile_pool(name="wpp", bufs=1))
        wpj = wpp.tile([P, 4 * C], F32R, tag="wpj")
        nc.sync.dma_start(
            wpj[:].rearrange("r (pr j) -> r pr j", pr=4),
            wproj_d[:].rearrange("(pr r) j -> r pr j", r=P),
        )

        # ---------------- Phase 2+3: attention + interleaved projection ----
        with ExitStack() as ph2:
            atp = ph2.enter_context(tc.tile_pool(name="atp", bufs=2))
            zrp = ph2.enter_context(tc.tile_pool(name="zrp", bufs=2))
            zsp_p = ph2.enter_context(tc.tile_pool(name="zsp_p", bufs=2))
            rzap = ph2.enter_context(tc.tile_pool(name="rzap", bufs=2))
            bcsp = ph2.enter_context(tc.tile_pool(name="bcsp", bufs=3))
            stgp = ph2.enter_context(tc.tile_pool(name="stgp", bufs=3))
            ysp = ph2.enter_context(tc.tile_pool(name="ysp", bufs=3))
            ps_s = ph2.enter_context(tc.tile_pool(name="ps_s", bufs=3, space="PSUM"))
            ps_o = ph2.enter_context(tc.tile_pool(name="ps_o", bufs=1, space="PSUM"))
            ps_b = ph2.enter_context(tc.tile_pool(name="ps_b", bufs=1, space="PSUM"))
            ps_y = ph2.enter_context(tc.tile_pool(name="ps_y", bufs=2, space="PSUM"))

            def emit_round_tail(qc, zra, zrb):
                # spread Z rows across 128 partitions, reciprocal, unspread
                zsp = zsp_p.tile([P, 32], F32, tag="zsp")
                for pr in range(4):
                    for hh in range(2):
                        r = pr * 2 + hh
                        srcz = (zra if hh == 0 else zrb)[pr * 32 : pr * 32 + 1, :]
                        nc.sync.dma_start(zsp[r * 16 : (r + 1) * 16, :], srcz)
                zspr = zsp_p.tile([P, 32], F32, tag="zspr")
                nc.vector.reciprocal(zspr[:], zsp[:])
                rz2 = rzap.tile([P, 512], F32R, tag="rz2")
                for pr in range(4):
                    for hh in range(2):
                        r = pr * 2 + hh
                        nc.sync.dma_start(
                            rz2[pr * 32 + hh : pr * 32 + hh + 1, :].bitcast(F32),
                            zspr[r * 16 : (r + 1) * 16, :],
                        )
                for pr in range(4):
                    col = pr * T + qc * 512
                    pbt = ps_b.tile([P, 512], F32, tag="pbt")
                    nc.tensor.matmul(
                        pbt[:],
                        sel2[pr * 32 : pr * 32 + 2, :],
                        rz2[pr * 32 : pr * 32 + 2, :],
                        start=True, stop=True,
                        tile_position=(pr * 32, 0),
                    )
                    bcs = bcsp.tile([P, 512], F32, tag="bcs")
                    nc.vector.tensor_copy(bcs[:], pbt[:])
                    nc.vector.tensor_mul(
                        aoT[0:64, col : col + 512],
                        aoT[0:64, col : col + 512],
                        bcs[0:64, :],
                    )
                    nc.vector.tensor_mul(
                        aoT[64:P, col : col + 512],
                        aoT[64:P, col : col + 512],
                        bcs[64:P, :],
                    )

            def emit_proj_round(qc):
                for tb in range(qc * 4, (qc + 1) * 4):
                    for oc in range(2):
                        py = ps_y.tile([P, 512], F32, tag="py")
                        for pp in range(4):
                            nc.tensor.matmul(
                                py[:],
                                aoT[:, pp * T + tb * P : pp * T + (tb + 1) * P],
                                wpj[:, pp * C + oc * 512 : pp * C + (oc + 1) * 512],
                                start=(pp == 0),
                                stop=(pp == 3),
                            )
                        ys = ysp.tile([P, 512], F32, tag="ys")
                        nc.vector.tensor_copy(ys[:], py[:])
                        nc.sync.dma_start(
                            y_d[tb * P : (tb + 1) * P, oc * 512 : (oc + 1) * 512],
                            ys[:],
                        )

            pending = None
            for qc in range(NQC):
                zra = zrp.tile([P, 512], F32, tag="zra")
                zrb = zrp.tile([P, 512], F32, tag="zrb")
                for p_pair in range(4):
                    kblk = 3 * p_pair + 2
                    po0 = ps_o.tile([65, 512], F32, tag="po0")
                    po1 = ps_o.tile([65, 512], F32, tag="po1")
                    po = [po0, po1]
                    nkb = 4 * qc + 4
                    for kb in range(nkb):
                        qoff = max(0, kb * P - qc * 512)
                        for hh in range(2):
                            qblk = 3 * p_pair + hh
                            ps = ps_s.tile([P, 512], F32, tag="ps")
                            nc.tensor.matmul(
                                ps[:, qoff:512],
                                qk_all[:, kblk * T + kb * P : kblk * T + (kb + 1) * P],
                                qk_all[:, qblk * T + qc * 512 + qoff : qblk * T + (qc + 1) * 512],
                                start=True,
                                stop=True,
                            )
                            at = atp.tile([P, 512], BF16, tag=f"at{hh}")
                            nc.scalar.activation(
                                at[:, qoff:512], ps[:, qoff:512], Exp, scale=SCALE,
                            )
                            if kb * P >= qc * 512:
                                # diagonal block: zero out k > q entries
                                nc.vector.tensor_mul(
                                    at[:, qoff : qoff + P],
                                    at[:, qoff : qoff + P],
                                    mask[:],
                                )
                            nc.tensor.matmul(
                                po[hh][:, qoff:512],
                                v_all[:, p_pair * NKB * 130 + kb * 130 + hh * 65 :
                                      p_pair * NKB * 130 + kb * 130 + hh * 65 + 65],
                                at[:, qoff:512],
                                start=(kb == 0),
                                stop=(kb == nkb - 1),
                                skip_group_check=True,
                            )
                    # evict raw ao + Z rows; normalization deferred one round
                    col = p_pair * T + qc * 512
                    nc.vector.tensor_copy(aoT[0:64, col : col + 512], po0[0:64, :])
                    stg = stgp.tile([64, 512], F32R, tag="stg")
                    nc.vector.tensor_copy(stg[:], po1[0:64, :])
                    nc.sync.dma_start(aoT[64:P, col : col + 512], stg[:])
                    nc.vector.tensor_copy(
                        zra[p_pair * 32 : p_pair * 32 + 1, :], po0[64:65, :]
                    )
                    nc.vector.tensor_copy(
                        zrb[p_pair * 32 : p_pair * 32 + 1, :], po1[64:65, :]
                    )
                if pending is not None:
                    emit_round_tail(*pending)
                    emit_proj_round(pending[0])
                pending = (qc, zra, zrb)
            emit_round_tail(*pending)
            emit_proj_round(pending[0])

    nc.compile()
    return nc


def _shard_inputs(x, W_qkv, b_qkv, W_proj):
    """Build the 8 per-core input maps."""
    in_maps = []
    for c in range(8):
        b = c // 2
        hg = c % 2
        heads = [hg * 8 + j for j in range(8)]
        qk_cols = []
        for p in range(4):
            ha, hb = heads[2 * p], heads[2 * p + 1]
            for part in range(2):  # q, k
                qk_cols.extend(range(ha * 192 + part * 64, ha * 192 + part * 64 + 64))
                qk_cols.extend(range(hb * 192 + part * 64, hb * 192 + part * 64 + 64))
        qk_cols = np.array(qk_cols)
        v_cols = []
        for p in range(4):
            ha, hb = heads[2 * p], heads[2 * p + 1]
            v_cols.extend(range(ha * 192 + 128, ha * 192 + 192))
            v_cols.extend(range(hb * 192 + 128, hb * 192 + 192))
        v_cols = np.array(v_cols)
        in_maps.append(
            {
                "xt": np.ascontiguousarray(x[b].T, dtype=np.float32),
                "wqk": np.ascontiguousarray(W_qkv[:, qk_cols], dtype=np.float32),
                "bqk": np.ascontiguousarray(b_qkv[qk_cols], dtype=np.float32),
                "wv": np.ascontiguousarray(W_qkv[:, v_cols], dtype=np.float32),
                "bv": np.ascontiguousarray(
                    b_qkv[v_cols].reshape(1, 512), dtype=np.float32
                ),
                "wproj": np.ascontiguousarray(
                    W_proj[hg * 512 : (hg + 1) * 512, :], dtype=np.float32
                ),
            }
        )
    return in_maps


_NC = None


def kernel(x, W_qkv, b_qkv, W_proj, b_proj, _trace=False):
    global _NC
    x = np.asarray(x, dtype=np.float32)
    W_qkv = np.asarray(W_qkv, dtype=np.float32)
    b_qkv = np.asarray(b_qkv, dtype=np.float32)
    W_proj = np.asarray(W_proj, dtype=np.float32)
    b_proj = np.asarray(b_proj, dtype=np.float32)

    in_maps = _shard_inputs(x, W_qkv, b_qkv, W_proj)
    if _NC is None:
        _NC = build_kernel()
    res = run_bass_kernel_spmd(
        _NC, in_maps, core_ids=list(range(8)), trace=_trace,
        trace_cores=list(range(8)) if _trace else None,
    )
    out = np.empty((B, T, C), dtype=np.float32)
    for b in range(B):
        out[b] = res.results[2 * b]["y"] + res.results[2 * b + 1]["y"] + b_proj
    if _trace:
        return out, res
    return out


# revision 23
# speedup vs baseline: 1.7516x; 1.0130x over previous
"""Multi-head causal self-attention (B=4, T=2048, C=1024, H=16) on 8 TRN2 cores.

Sharding: core c handles batch b = c//2 and head-group hg = c%2 (8 heads):
data parallel over B, tensor parallel over H. Each core computes qk^T for its
heads (xT @ Wqk column-slice, transposed per-head-pair layout), V in natural
layout, causal attention for its 8 heads, and a partial output projection
(row-split W_proj) -> y_partial [T, C]. Host transposes x per core and sums
y[b] = y_partial[2b] + y_partial[2b+1] + b_proj.

Matmul dtypes: q/k in bf16 (scores accumulate in fp32 PSUM), everything else
float32r (TF32-like). Scores are computed transposed ([k, q]) with
zero-padded q copies so every matmul is full-K at partition base 0. The
softmax denominator arrives via a ones-row folded into the attn@V matmul;
normalization uses a fast approximate reciprocal + K=1 broadcast matmul.
"""

from contextlib import ExitStack

import numpy as np

import concourse.bass as bass
import concourse.bacc as bacc
import concourse.mybir as mybir
import concourse.tile as tile
from concourse.bass_utils import run_bass_kernel_spmd
from concourse.masks import make_upper_triangular

B, T, C, H, HS = 4, 2048, 1024, 16, 64
P = 128
NQC = T // 512          # q-chunks of 512
NKB = T // P            # key blocks of 128
NTB = T // P            # t blocks of 128
TH = T // 2             # t-half
SCALE = HS ** -0.5

F32 = mybir.dt.float32
F32R = mybir.dt.float32r
BF16 = mybir.dt.bfloat16
Exp = mybir.ActivationFunctionType.Exp


def build_kernel():
    nc = bacc.Bacc("TRN2", target_bir_lowering=False)

    xt_d = nc.dram_tensor("xt", (C, T), F32R, kind="ExternalInput")
    wqk_d = nc.dram_tensor("wqk", (C, 8 * P), F32R, kind="ExternalInput")
    bqk_d = nc.dram_tensor("bqk", (8 * P,), F32, kind="ExternalInput")
    wv_d = nc.dram_tensor("wv", (C, 512), F32R, kind="ExternalInput")
    bv_d = nc.dram_tensor("bv", (1, 512), F32R, kind="ExternalInput")
    wproj_d = nc.dram_tensor("wproj", (8 * HS, C), F32R, kind="ExternalInput")
    y_d = nc.dram_tensor("y", (T, C), F32, kind="ExternalOutput")

    with tile.TileContext(nc) as tc, ExitStack() as big:
        const = big.enter_context(tc.tile_pool(name="const", bufs=1))
        persist = big.enter_context(tc.tile_pool(name="persist", bufs=1))

        # mask[k, q] = 1 where k <= q (valid causal entries of a diag block)
        mask = const.tile([P, P], BF16, tag="mask")
        make_upper_triangular(nc, mask[:], val=1.0, diag=True)
        ones_f = const.tile([P, P], F32, tag="ones_f")
        nc.vector.memset(ones_f[:], 1.0)
        ones_t = const.tile([1, P], F32R, tag="ones")
        nc.vector.tensor_copy(ones_t[:], ones_f[0:1, :])

        # qk_all: 12 blocks of [128, T] bf16; per pair p:
        #   block 3p   = qpadA: rows 0:64 q of head 2p, rows 64:128 zero
        #   block 3p+1 = qpadB: rows 0:64 zero, rows 64:128 q of head 2p+1
        #   block 3p+2 = k pair: rows 0:64 k(2p), 64:128 k(2p+1)
        qk_all = persist.tile([P, 12 * T], BF16, tag="qk")
        for p_pair in range(4):
            nc.vector.memset(qk_all[64:P, (3 * p_pair) * T : (3 * p_pair + 1) * T], 0.0)
            nc.vector.memset(qk_all[0:64, (3 * p_pair + 1) * T : (3 * p_pair + 2) * T], 0.0)

        # v_all: per (pair, kb): [vA(64) | onesA(1) | vB(64) | onesB(1)] = 130
        v_all = persist.tile([P, 4 * NKB * 130], BF16, tag="v")
        va4 = v_all[:].rearrange("p (a b c) -> p a b c", a=4, b=NKB, c=130)
        nc.vector.tensor_copy(va4[:, :, :, 64:65], ones_f[:, 0 : 4 * NKB])
        nc.vector.tensor_copy(va4[:, :, :, 129:130], ones_f[:, 0 : 4 * NKB])

        # ---------------- Phase 1: qk^T, natural V ----------------
        with ExitStack() as ph1:
            cp1 = ph1.enter_context(tc.tile_pool(name="cp1", bufs=1))
            xtp = ph1.enter_context(tc.tile_pool(name="xtp", bufs=2))
            wp = ph1.enter_context(tc.tile_pool(name="wp", bufs=2))
            ps_m = ph1.enter_context(tc.tile_pool(name="ps_m", bufs=3, space="PSUM"))
            ps_v = ph1.enter_context(tc.tile_pool(name="ps_v", bufs=3, space="PSUM"))
            ps_c = ph1.enter_context(tc.tile_pool(name="ps_c", bufs=1, space="PSUM"))

            bqk = cp1.tile([P, 8], F32, tag="bqk")
            nc.sync.dma_start(bqk[:], bqk_d[:].rearrange("(a p) -> p a", p=P))
            wv_sb = cp1.tile([P, 8 * 512], F32R, tag="wv")
            nc.sync.dma_start(
                wv_sb[:].rearrange("p (cb j) -> p cb j", cb=8),
                wv_d[:].rearrange("(cb p) j -> p cb j", p=P),
            )
            # bias_v[128, 512] = b_v broadcast along partitions (K=1 matmul)
            bvr = cp1.tile([1, 512], F32R, tag="bvr")
            nc.sync.dma_start(bvr[:], bv_d[:])
            bias_v = cp1.tile([P, 512], F32, tag="bias_v")
            pbv = ps_c.tile([P, 512], F32, tag="pbv")
            nc.tensor.matmul(pbv[:], ones_t[:], bvr[:], start=True, stop=True)
            nc.vector.tensor_copy(bias_v[:], pbv[:])

            for th in range(2):  # t-half
                xT = xtp.tile([P, 8 * TH], F32R, tag="xT")
                for cb in range(8):
                    nc.sync.dma_start(
                        xT[:, cb * TH : (cb + 1) * TH],
                        xt_d[cb * P : (cb + 1) * P, th * TH : (th + 1) * TH],
                    )
                # q, k projections (transposed layout)
                for chb in range(8):
                    p_pair = chb // 2
                    kind = chb % 2  # 0 = q block, 1 = k block
                    wb = wp.tile([P, 8 * P], F32R, tag="w")
                    nc.sync.dma_start(
                        wb[:].rearrange("p (cb j) -> p cb j", cb=8),
                        wqk_d[:, chb * P : (chb + 1) * P].rearrange(
                            "(cb p) j -> p cb j", p=P
                        ),
                    )
                    for tck in range(2):
                        pq = ps_m.tile([P, 512], F32, tag="pq")
                        for cb in range(8):
                            nc.tensor.matmul(
                                pq[:],
                                wb[:, cb * P : (cb + 1) * P],
                                xT[:, cb * TH + tck * 512 : cb * TH + (tck + 1) * 512],
                                start=(cb == 0),
                                stop=(cb == 7),
                            )
                        t0 = th * TH + tck * 512
                        if kind == 0:  # q -> two zero-padded tiles
                            blk_a, blk_b = 3 * p_pair, 3 * p_pair + 1
                            nc.vector.tensor_scalar_add(
                                qk_all[0:64, blk_a * T + t0 : blk_a * T + t0 + 512],
                                pq[0:64, :],
                                bqk[0:64, chb : chb + 1],
                            )
                            nc.vector.tensor_scalar_add(
                                qk_all[64:P, blk_b * T + t0 : blk_b * T + t0 + 512],
                                pq[64:P, :],
                                bqk[64:P, chb : chb + 1],
                            )
                        else:  # k pair block
                            blk = 3 * p_pair + 2
                            nc.vector.tensor_scalar_add(
                                qk_all[:, blk * T + t0 : blk * T + t0 + 512],
                                pq[:],
                                bqk[:, chb : chb + 1],
                            )
                # natural-layout V (xT stationary, wv moving)
                for tb in range(8):
                    kb = th * 8 + tb
                    pv = ps_v.tile([P, 512], F32, tag="pv")
                    for cb in range(8):
                        nc.tensor.matmul(
                            pv[:],
                            xT[:, cb * TH + tb * P : cb * TH + (tb + 1) * P],
                            wv_sb[:, cb * 512 : (cb + 1) * 512],
                            start=(cb == 0),
                            stop=(cb == 7),
                        )
                    dst = bass.AP(
                        v_all[:].tensor,
                        v_all[:].offset + kb * 130,
                        [[v_all[:].ap[0][0], P], [NKB * 130, 4], [65, 2], [1, 64]],
                    )
                    src = bass.AP(
                        pv[:].tensor,
                        pv[:].offset,
                        [[pv[:].ap[0][0], P], [128, 4], [64, 2], [1, 64]],
                    )
                    bsrc = bass.AP(
                        bias_v[:].tensor,
                        bias_v[:].offset,
                        [[bias_v[:].ap[0][0], P], [128, 4], [64, 2], [1, 64]],
                    )
                    nc.vector.tensor_tensor(dst, src, bsrc, mybir.AluOpType.add)

        # aoT: pair-stacked [128 = ch(head 2p) | ch(head 2p+1), 4 * T]
        persist2 = big.enter_context(tc.tile_pool(name="persist2", bufs=1))
        aoT = persist2.tile([P, 4 * T], F32R, tag="aoT")

        # sel2: rows {32p: cols 0:64 = 1}, {32p+1: cols 64:128 = 1}, else 0
        sel2 = const.tile([P, P], F32R, tag="sel2")
        nc.vector.memset(sel2[:].bitcast(F32), 0.0)
        for pr in range(4):
            nc.sync.dma_start(sel2[pr * 32 : pr * 32 + 1, 0:64].bitcast(F32), ones_f[0:1, 0:64])
            nc.sync.dma_start(
                sel2[pr * 32 + 1 : pr * 32 + 2, 64:P].bitcast(F32), ones_f[0:1, 0:64]
            )

        # wproj prefetch (DMA overlaps with attention)
        wpp = big.enter_context(tc.tile_pool(name="wpp", bufs=1))
        wpj = wpp.tile([P, 4 * C], F32R, tag="wpj")
        nc.sync.dma_start(
            wpj[:].rearrange("r (pr j) -> r pr j", pr=4),
            wproj_d[:].rearrange("(pr r) j -> r pr j", r=P),
        )

        # ---------------- Phase 2+3: attention + interleaved projection ----
        with ExitStack() as ph2:
            atp = ph2.enter_context(tc.tile_pool(name="atp", bufs=2))
            zrp = ph2.enter_context(tc.tile_pool(name="zrp", bufs=2))
            zsp_p = ph2.enter_context(tc.tile_pool(name="zsp_p", bufs=2))
            rzap = ph2.enter_context(tc.tile_pool(name="rzap", bufs=2))
            bcsp = ph2.enter_context(tc.tile_pool(name="bcsp", bufs=3))
            stgp = ph2.enter_context(tc.tile_pool(name="stgp", bufs=3))
            ysp = ph2.enter_context(tc.tile_pool(name="ysp", bufs=3))
            ps_s = ph2.enter_context(tc.tile_pool(name="ps_s", bufs=3, space="PSUM"))
            ps_o = ph2.enter_context(tc.tile_pool(name="ps_o", bufs=1, space="PSUM"))
            ps_b = ph2.enter_context(tc.tile_pool(name="ps_b", bufs=1, space="PSUM"))
            ps_y = ph2.enter_context(tc.tile_pool(name="ps_y", bufs=2, space="PSUM"))

            def emit_round_tail(qc, zra, zrb):
                # spread Z rows across 128 partitions, reciprocal, unspread
                zsp = zsp_p.tile([P, 32], F32, tag="zsp")
                for pr in range(4):
                    for hh in range(2):
                        r = pr * 2 + hh
                        srcz = (zra if hh == 0 else zrb)[pr * 32 : pr * 32 + 1, :]
                        nc.sync.dma_start(zsp[r * 16 : (r + 1) * 16, :], srcz)
                zspr = zsp_p.tile([P, 32], F32, tag="zspr")
                nc.vector.reciprocal(zspr[:], zsp[:])
                rz2 = rzap.tile([P, 512], F32R, tag="rz2")
                for pr in range(4):
                    for hh in range(2):
                        r = pr * 2 + hh
                        nc.sync.dma_start(
                            rz2[pr * 32 + hh : pr * 32 + hh + 1, :].bitcast(F32),
                            zspr[r * 16 : (r + 1) * 16, :],
                        )
                for pr in range(4):
                    col = pr * T + qc * 512
                    pbt = ps_b.tile([P, 512], F32, tag="pbt")
                    nc.tensor.matmul(
                        pbt[:],
                        sel2[pr * 32 : pr * 32 + 2, :],
                        rz2[pr * 32 : pr * 32 + 2, :],
                        start=True, stop=True,
                        tile_position=(pr * 32, 0),
                    )
                    nc.vector.tensor_mul(
                        aoT[0:64, col : col + 512],
                        aoT[0:64, col : col + 512],
                        pbt[0:64, :],
                    )
                    nc.vector.tensor_mul(
                        aoT[64:P, col : col + 512],
                        aoT[64:P, col : col + 512],
                        pbt[64:P, :],
                    )

            def emit_proj_round(qc):
                for tb in range(qc * 4, (qc + 1) * 4):
                    for oc in range(2):
                        py = ps_y.tile([P, 512], F32, tag="py")
                        for pp in range(4):
                            nc.tensor.matmul(
                                py[:],
                                aoT[:, pp * T + tb * P : pp * T + (tb + 1) * P],
                                wpj[:, pp * C + oc * 512 : pp * C + (oc + 1) * 512],
                                start=(pp == 0),
                                stop=(pp == 3),
                            )
                        ys = ysp.tile([P, 512], F32, tag="ys")
                        nc.vector.tensor_copy(ys[:], py[:])
                        nc.sync.dma_start(
                            y_d[tb * P : (tb + 1) * P, oc * 512 : (oc + 1) * 512],
                            ys[:],
                        )

            pending = None
            for qc in range(NQC):
                zra = zrp.tile([P, 512], F32, tag="zra")
                zrb = zrp.tile([P, 512], F32, tag="zrb")
                for p_pair in range(4):
                    kblk = 3 * p_pair + 2
                    po0 = ps_o.tile([65, 512], F32, tag="po0")
                    po1 = ps_o.tile([65, 512], F32, tag="po1")
                    po = [po0, po1]
                    nkb = 4 * qc + 4
                    for kb in range(nkb):
                        qoff = max(0, kb * P - qc * 512)
                        ats = []
                        for hh in range(2):
                            qblk = 3 * p_pair + hh
                            ps = ps_s.tile([P, 512], F32, tag="ps")
                            nc.tensor.matmul(
                                ps[:, qoff:512],
                                qk_all[:, kblk * T + kb * P : kblk * T + (kb + 1) * P],
                                qk_all[:, qblk * T + qc * 512 + qoff : qblk * T + (qc + 1) * 512],
                                start=True,
                                stop=True,
                            )
                            at = atp.tile([P, 512], BF16, tag=f"at{hh}")
                            nc.scalar.activation(
                                at[:, qoff:512], ps[:, qoff:512], Exp, scale=SCALE,
                            )
                            if kb * P >= qc * 512:
                                # diagonal block: zero out k > q entries
                                nc.vector.tensor_mul(
                                    at[:, qoff : qoff + P],
                                    at[:, qoff : qoff + P],
                                    mask[:],
                                )
                            ats.append(at)
                        for hh in range(2):
                            nc.tensor.matmul(
                                po[hh][:, qoff:512],
                                v_all[:, p_pair * NKB * 130 + kb * 130 + hh * 65 :
                                      p_pair * NKB * 130 + kb * 130 + hh * 65 + 65],
                                ats[hh][:, qoff:512],
                                start=(kb == 0),
                                stop=(kb == nkb - 1),
                                skip_group_check=True,
                            )
                    # evict raw ao + Z rows; normalization deferred one round
                    col = p_pair * T + qc * 512
                    nc.vector.tensor_copy(aoT[0:64, col : col + 512], po0[0:64, :])
                    nc.vector.tensor_copy(aoT[64:P, col : col + 512], po1[0:64, :])
                    nc.vector.tensor_copy(
                        zra[p_pair * 32 : p_pair * 32 + 1, :], po0[64:65, :]
                    )
                    nc.vector.tensor_copy(
                        zrb[p_pair * 32 : p_pair * 32 + 1, :], po1[64:65, :]
                    )
                if pending is not None:
                    emit_round_tail(*pending)
                    emit_proj_round(pending[0])
                pending = (qc, zra, zrb)
            emit_round_tail(*pending)
            emit_proj_round(pending[0])

    nc.compile()
    return nc


def _shard_inputs(x, W_qkv, b_qkv, W_proj):
    """Build the 8 per-core input maps."""
    in_maps = []
    for c in range(8):
        b = c // 2
        hg = c % 2
        heads = [hg * 8 + j for j in range(8)]
        qk_cols = []
        for p in range(4):
            ha, hb = heads[2 * p], heads[2 * p + 1]
            for part in range(2):  # q, k
                qk_cols.extend(range(ha * 192 + part * 64, ha * 192 + part * 64 + 64))
                qk_cols.extend(range(hb * 192 + part * 64, hb * 192 + part * 64 + 64))
        qk_cols = np.array(qk_cols)
        v_cols = []
        for p in range(4):
            ha, hb = heads[2 * p], heads[2 * p + 1]
            v_cols.extend(range(ha * 192 + 128, ha * 192 + 192))
            v_cols.extend(range(hb * 192 + 128, hb * 192 + 192))
        v_cols = np.array(v_cols)
        in_maps.append(
            {
                "xt": np.ascontiguousarray(x[b].T, dtype=np.float32),
                "wqk": np.ascontiguousarray(W_qkv[:, qk_cols], dtype=np.float32),
                "bqk": np.ascontiguousarray(b_qkv[qk_cols], dtype=np.float32),
                "wv": np.ascontiguousarray(W_qkv[:, v_cols], dtype=np.float32),
                "bv": np.ascontiguousarray(
                    b_qkv[v_cols].reshape(1, 512), dtype=np.float32
                ),
                "wproj": np.ascontiguousarray(
                    W_proj[hg * 512 : (hg + 1) * 512, :], dtype=np.float32
                ),
            }
        )
    return in_maps


_NC = None


def kernel(x, W_qkv, b_qkv, W_proj, b_proj, _trace=False):
    global _NC
    x = np.asarray(x, dtype=np.float32)
    W_qkv = np.asarray(W_qkv, dtype=np.float32)
    b_qkv = np.asarray(b_qkv, dtype=np.float32)
    W_proj = np.asarray(W_proj, dtype=np.float32)
    b_proj = np.asarray(b_proj, dtype=np.float32)

    in_maps = _shard_inputs(x, W_qkv, b_qkv, W_proj)
    if _NC is None:
        _NC = build_kernel()
    res = run_bass_kernel_spmd(
        _NC, in_maps, core_ids=list(range(8)), trace=_trace,
        trace_cores=list(range(8)) if _trace else None,
    )
    out = np.empty((B, T, C), dtype=np.float32)
    for b in range(B):
        out[b] = res.results[2 * b]["y"] + res.results[2 * b + 1]["y"] + b_proj
    if _trace:
        return out, res
    return out


# revision 24
# speedup vs baseline: 1.7827x; 1.0178x over previous
"""Multi-head causal self-attention (B=4, T=2048, C=1024, H=16) on 8 TRN2 cores.

Sharding: core c handles batch b = c//2 and head-group hg = c%2 (8 heads):
data parallel over B, tensor parallel over H. Each core computes qk^T for its
heads (xT @ Wqk column-slice, transposed per-head-pair layout), V in natural
layout, causal attention for its 8 heads, and a partial output projection
(row-split W_proj) -> y_partial [T, C]. Host transposes x per core and sums
y[b] = y_partial[2b] + y_partial[2b+1] + b_proj.

Matmul dtypes: q/k in bf16 (scores accumulate in fp32 PSUM), everything else
float32r (TF32-like). Scores are computed transposed ([k, q]) with
zero-padded q copies so every matmul is full-K at partition base 0. The
softmax denominator arrives via a ones-row folded into the attn@V matmul;
normalization uses a fast approximate reciprocal + K=1 broadcast matmul.
"""

from contextlib import ExitStack

import numpy as np

import concourse.bass as bass
import concourse.bacc as bacc
import concourse.mybir as mybir
import concourse.tile as tile
from concourse.bass_utils import run_bass_kernel_spmd
from concourse.masks import make_upper_triangular

B, T, C, H, HS = 4, 2048, 1024, 16, 64
P = 128
NQC = T // 512          # q-chunks of 512
NKB = T // P            # key blocks of 128
NTB = T // P            # t blocks of 128
TH = T // 2             # t-half
SCALE = HS ** -0.5

F32 = mybir.dt.float32
F32R = mybir.dt.float32r
BF16 = mybir.dt.bfloat16
Exp = mybir.ActivationFunctionType.Exp


def build_kernel():
    nc = bacc.Bacc("TRN2", target_bir_lowering=False)

    xt_d = nc.dram_tensor("xt", (C, T), F32R, kind="ExternalInput")
    wqk_d = nc.dram_tensor("wqk", (C, 8 * P), F32R, kind="ExternalInput")
    bqk_d = nc.dram_tensor("bqk", (8 * P,), F32, kind="ExternalInput")
    wv_d = nc.dram_tensor("wv", (C, 512), F32R, kind="ExternalInput")
    bv_d = nc.dram_tensor("bv", (1, 512), F32R, kind="ExternalInput")
    wproj_d = nc.dram_tensor("wproj", (8 * HS, C), F32R, kind="ExternalInput")
    y_d = nc.dram_tensor("y", (T, C), F32, kind="ExternalOutput")

    with tile.TileContext(nc) as tc, ExitStack() as big:
        const = big.enter_context(tc.tile_pool(name="const", bufs=1))
        persist = big.enter_context(tc.tile_pool(name="persist", bufs=1))

        # mask[k, q] = 1 where k <= q (valid causal entries of a diag block)
        mask = const.tile([P, P], BF16, tag="mask")
        make_upper_triangular(nc, mask[:], val=1.0, diag=True)
        ones_f = const.tile([P, P], F32, tag="ones_f")
        nc.vector.memset(ones_f[:], 1.0)
        ones_t = const.tile([1, P], F32R, tag="ones")
        nc.vector.tensor_copy(ones_t[:], ones_f[0:1, :])

        # qk_all: 12 blocks of [128, T] bf16; per pair p:
        #   block 3p   = qpadA: rows 0:64 q of head 2p, rows 64:128 zero
        #   block 3p+1 = qpadB: rows 0:64 zero, rows 64:128 q of head 2p+1
        #   block 3p+2 = k pair: rows 0:64 k(2p), 64:128 k(2p+1)
        qk_all = persist.tile([P, 12 * T], BF16, tag="qk")
        for p_pair in range(4):
            nc.vector.memset(qk_all[64:P, (3 * p_pair) * T : (3 * p_pair + 1) * T], 0.0)
            nc.vector.memset(qk_all[0:64, (3 * p_pair + 1) * T : (3 * p_pair + 2) * T], 0.0)

        # v_all: per (pair, kb): [vA(64) | onesA(1) | vB(64) | onesB(1)] = 130
        v_all = persist.tile([P, 4 * NKB * 130], BF16, tag="v")
        va4 = v_all[:].rearrange("p (a b c) -> p a b c", a=4, b=NKB, c=130)
        nc.vector.tensor_copy(va4[:, :, :, 64:65], ones_f[:, 0 : 4 * NKB])
        nc.vector.tensor_copy(va4[:, :, :, 129:130], ones_f[:, 0 : 4 * NKB])

        # ---------------- Phase 1: qk^T, natural V ----------------
        with ExitStack() as ph1:
            cp1 = ph1.enter_context(tc.tile_pool(name="cp1", bufs=1))
            xtp = ph1.enter_context(tc.tile_pool(name="xtp", bufs=2))
            wp = ph1.enter_context(tc.tile_pool(name="wp", bufs=2))
            ps_m = ph1.enter_context(tc.tile_pool(name="ps_m", bufs=3, space="PSUM"))
            ps_v = ph1.enter_context(tc.tile_pool(name="ps_v", bufs=3, space="PSUM"))
            ps_c = ph1.enter_context(tc.tile_pool(name="ps_c", bufs=1, space="PSUM"))

            bqk = cp1.tile([P, 8], F32, tag="bqk")
            nc.sync.dma_start(bqk[:], bqk_d[:].rearrange("(a p) -> p a", p=P))
            wv_sb = cp1.tile([P, 8 * 512], F32R, tag="wv")
            nc.sync.dma_start(
                wv_sb[:].rearrange("p (cb j) -> p cb j", cb=8),
                wv_d[:].rearrange("(cb p) j -> p cb j", p=P),
            )
            # bias_v[128, 512] = b_v broadcast along partitions (K=1 matmul)
            bvr = cp1.tile([1, 512], F32R, tag="bvr")
            nc.sync.dma_start(bvr[:], bv_d[:])
            bias_v = cp1.tile([P, 512], F32, tag="bias_v")
            pbv = ps_c.tile([P, 512], F32, tag="pbv")
            nc.tensor.matmul(pbv[:], ones_t[:], bvr[:], start=True, stop=True)
            nc.vector.tensor_copy(bias_v[:], pbv[:])

            for th in range(2):  # t-half
                xT = xtp.tile([P, 8 * TH], F32R, tag="xT")
                for cb in range(8):
                    nc.sync.dma_start(
                        xT[:, cb * TH : (cb + 1) * TH],
                        xt_d[cb * P : (cb + 1) * P, th * TH : (th + 1) * TH],
                    )
                # q, k projections (transposed layout)
                for chb in range(8):
                    p_pair = chb // 2
                    kind = chb % 2  # 0 = q block, 1 = k block
                    wb = wp.tile([P, 8 * P], F32R, tag="w")
                    nc.sync.dma_start(
                        wb[:].rearrange("p (cb j) -> p cb j", cb=8),
                        wqk_d[:, chb * P : (chb + 1) * P].rearrange(
                            "(cb p) j -> p cb j", p=P
                        ),
                    )
                    for tck in range(2):
                        pq = ps_m.tile([P, 512], F32, tag="pq")
                        for cb in range(8):
                            nc.tensor.matmul(
                                pq[:],
                                wb[:, cb * P : (cb + 1) * P],
                                xT[:, cb * TH + tck * 512 : cb * TH + (tck + 1) * 512],
                                start=(cb == 0),
                                stop=(cb == 7),
                            )
                        t0 = th * TH + tck * 512
                        if kind == 0:  # q -> two zero-padded tiles
                            blk_a, blk_b = 3 * p_pair, 3 * p_pair + 1
                            nc.vector.tensor_scalar_add(
                                qk_all[0:64, blk_a * T + t0 : blk_a * T + t0 + 512],
                                pq[0:64, :],
                                bqk[0:64, chb : chb + 1],
                            )
                            nc.vector.tensor_scalar_add(
                                qk_all[64:P, blk_b * T + t0 : blk_b * T + t0 + 512],
                                pq[64:P, :],
                                bqk[64:P, chb : chb + 1],
                            )
                        else:  # k pair block
                            blk = 3 * p_pair + 2
                            nc.vector.tensor_scalar_add(
                                qk_all[:, blk * T + t0 : blk * T + t0 + 512],
                                pq[:],
                                bqk[:, chb : chb + 1],
                            )
                # natural-layout V (xT stationary, wv moving)
                for tb in range(8):
                    kb = th * 8 + tb
                    pv = ps_v.tile([P, 512], F32, tag="pv")
                    for cb in range(8):
                        nc.tensor.matmul(
                            pv[:],
                            xT[:, cb * TH + tb * P : cb * TH + (tb + 1) * P],
                            wv_sb[:, cb * 512 : (cb + 1) * 512],
                            start=(cb == 0),
                            stop=(cb == 7),
                        )
                    dst = bass.AP(
                        v_all[:].tensor,
                        v_all[:].offset + kb * 130,
                        [[v_all[:].ap[0][0], P], [NKB * 130, 4], [65, 2], [1, 64]],
                    )
                    src = bass.AP(
                        pv[:].tensor,
                        pv[:].offset,
                        [[pv[:].ap[0][0], P], [128, 4], [64, 2], [1, 64]],
                    )
                    bsrc = bass.AP(
                        bias_v[:].tensor,
                        bias_v[:].offset,
                        [[bias_v[:].ap[0][0], P], [128, 4], [64, 2], [1, 64]],
                    )
                    nc.vector.tensor_tensor(dst, src, bsrc, mybir.AluOpType.add)

        # aoT: pair-stacked [128 = ch(head 2p) | ch(head 2p+1), 4 * T]
        persist2 = big.enter_context(tc.tile_pool(name="persist2", bufs=1))
        aoT = persist2.tile([P, 4 * T], F32R, tag="aoT")

        # sel2: rows {32p: cols 0:64 = 1}, {32p+1: cols 64:128 = 1}, else 0
        sel2 = const.tile([P, P], F32R, tag="sel2")
        nc.vector.memset(sel2[:].bitcast(F32), 0.0)
        for pr in range(4):
            nc.sync.dma_start(sel2[pr * 32 : pr * 32 + 1, 0:64].bitcast(F32), ones_f[0:1, 0:64])
            nc.sync.dma_start(
                sel2[pr * 32 + 1 : pr * 32 + 2, 64:P].bitcast(F32), ones_f[0:1, 0:64]
            )

        # wproj prefetch (DMA overlaps with attention)
        wpp = big.enter_context(tc.tile_pool(name="wpp", bufs=1))
        wpj = wpp.tile([P, 4 * C], F32R, tag="wpj")
        nc.sync.dma_start(
            wpj[:].rearrange("r (pr j) -> r pr j", pr=4),
            wproj_d[:].rearrange("(pr r) j -> r pr j", r=P),
        )

        # ---------------- Phase 2+3: attention + interleaved projection ----
        with ExitStack() as ph2:
            atp = ph2.enter_context(tc.tile_pool(name="atp", bufs=2))
            zrp = ph2.enter_context(tc.tile_pool(name="zrp", bufs=2))
            zsp_p = ph2.enter_context(tc.tile_pool(name="zsp_p", bufs=2))
            rzap = ph2.enter_context(tc.tile_pool(name="rzap", bufs=2))
            bcsp = ph2.enter_context(tc.tile_pool(name="bcsp", bufs=3))
            stgp = ph2.enter_context(tc.tile_pool(name="stgp", bufs=3))
            ysp = ph2.enter_context(tc.tile_pool(name="ysp", bufs=3))
            ps_s = ph2.enter_context(tc.tile_pool(name="ps_s", bufs=3, space="PSUM"))
            ps_o = ph2.enter_context(tc.tile_pool(name="ps_o", bufs=1, space="PSUM"))
            ps_b = ph2.enter_context(tc.tile_pool(name="ps_b", bufs=1, space="PSUM"))
            ps_y = ph2.enter_context(tc.tile_pool(name="ps_y", bufs=2, space="PSUM"))

            def emit_round_tail(qc, zra, zrb, prs=(0, 1, 2, 3)):
                # spread Z rows across 128 partitions, reciprocal, unspread
                lo = min(prs) * 32
                hi = (max(prs) + 1) * 32
                zsp = zsp_p.tile([P, 32], F32, tag="zsp")
                for pr in prs:
                    for hh in range(2):
                        r = pr * 2 + hh
                        srcz = (zra if hh == 0 else zrb)[pr * 32 : pr * 32 + 1, :]
                        nc.sync.dma_start(zsp[r * 16 : (r + 1) * 16, :], srcz)
                zspr = zsp_p.tile([P, 32], F32, tag="zspr")
                nc.vector.reciprocal(zspr[lo:hi, :], zsp[lo:hi, :])
                rz2 = rzap.tile([P, 512], F32R, tag="rz2")
                for pr in prs:
                    for hh in range(2):
                        r = pr * 2 + hh
                        nc.sync.dma_start(
                            rz2[pr * 32 + hh : pr * 32 + hh + 1, :].bitcast(F32),
                            zspr[r * 16 : (r + 1) * 16, :],
                        )
                for pr in prs:
                    col = pr * T + qc * 512
                    pbt = ps_b.tile([P, 512], F32, tag="pbt")
                    nc.tensor.matmul(
                        pbt[:],
                        sel2[pr * 32 : pr * 32 + 2, :],
                        rz2[pr * 32 : pr * 32 + 2, :],
                        start=True, stop=True,
                        tile_position=(pr * 32, 0),
                    )
                    nc.vector.tensor_mul(
                        aoT[0:64, col : col + 512],
                        aoT[0:64, col : col + 512],
                        pbt[0:64, :],
                    )
                    nc.vector.tensor_mul(
                        aoT[64:P, col : col + 512],
                        aoT[64:P, col : col + 512],
                        pbt[64:P, :],
                    )

            def emit_proj_round(qc):
                for tb in range(qc * 4, (qc + 1) * 4):
                    for oc in range(2):
                        py = ps_y.tile([P, 512], F32, tag="py")
                        for pp in range(4):
                            nc.tensor.matmul(
                                py[:],
                                aoT[:, pp * T + tb * P : pp * T + (tb + 1) * P],
                                wpj[:, pp * C + oc * 512 : pp * C + (oc + 1) * 512],
                                start=(pp == 0),
                                stop=(pp == 3),
                            )
                        ys = ysp.tile([P, 512], F32, tag="ys")
                        nc.vector.tensor_copy(ys[:], py[:])
                        nc.sync.dma_start(
                            y_d[tb * P : (tb + 1) * P, oc * 512 : (oc + 1) * 512],
                            ys[:],
                        )

            pending = None
            for qc in range(NQC):
                zra = zrp.tile([P, 512], F32, tag="zra")
                zrb = zrp.tile([P, 512], F32, tag="zrb")
                for p_pair in range(4):
                    kblk = 3 * p_pair + 2
                    po0 = ps_o.tile([65, 512], F32, tag="po0")
                    po1 = ps_o.tile([65, 512], F32, tag="po1")
                    po = [po0, po1]
                    nkb = 4 * qc + 4
                    for kb in range(nkb):
                        qoff = max(0, kb * P - qc * 512)
                        ats = []
                        for hh in range(2):
                            qblk = 3 * p_pair + hh
                            ps = ps_s.tile([P, 512], F32, tag="ps")
                            nc.tensor.matmul(
                                ps[:, qoff:512],
                                qk_all[:, kblk * T + kb * P : kblk * T + (kb + 1) * P],
                                qk_all[:, qblk * T + qc * 512 + qoff : qblk * T + (qc + 1) * 512],
                                start=True,
                                stop=True,
                            )
                            at = atp.tile([P, 512], BF16, tag=f"at{hh}")
                            nc.scalar.activation(
                                at[:, qoff:512], ps[:, qoff:512], Exp, scale=SCALE,
                            )
                            if kb * P >= qc * 512:
                                # diagonal block: zero out k > q entries
                                nc.vector.tensor_mul(
                                    at[:, qoff : qoff + P],
                                    at[:, qoff : qoff + P],
                                    mask[:],
                                )
                            ats.append(at)
                        for hh in range(2):
                            nc.tensor.matmul(
                                po[hh][:, qoff:512],
                                v_all[:, p_pair * NKB * 130 + kb * 130 + hh * 65 :
                                      p_pair * NKB * 130 + kb * 130 + hh * 65 + 65],
                                ats[hh][:, qoff:512],
                                start=(kb == 0),
                                stop=(kb == nkb - 1),
                                skip_group_check=True,
                            )
                    # evict raw ao + Z rows; normalization deferred one round
                    col = p_pair * T + qc * 512
                    nc.vector.tensor_copy(aoT[0:64, col : col + 512], po0[0:64, :])
                    nc.vector.tensor_copy(aoT[64:P, col : col + 512], po1[0:64, :])
                    nc.vector.tensor_copy(
                        zra[p_pair * 32 : p_pair * 32 + 1, :], po0[64:65, :]
                    )
                    nc.vector.tensor_copy(
                        zrb[p_pair * 32 : p_pair * 32 + 1, :], po1[64:65, :]
                    )
                    if qc == NQC - 1 and p_pair == 1:
                        emit_round_tail(qc, zra, zrb, prs=(0, 1))
                if pending is not None:
                    emit_round_tail(*pending)
                    emit_proj_round(pending[0])
                pending = (qc, zra, zrb)
            emit_round_tail(*pending, prs=(2, 3))
            emit_proj_round(pending[0])

    nc.compile()
    return nc


def _shard_inputs(x, W_qkv, b_qkv, W_proj):
    """Build the 8 per-core input maps."""
    in_maps = []
    for c in range(8):
        b = c // 2
        hg = c % 2
        heads = [hg * 8 + j for j in range(8)]
        qk_cols = []
        for p in range(4):
            ha, hb = heads[2 * p], heads[2 * p + 1]
            for part in range(2):  # q, k
                qk_cols.extend(range(ha * 192 + part * 64, ha * 192 + part * 64 + 64))
                qk_cols.extend(range(hb * 192 + part * 64, hb * 192 + part * 64 + 64))
        qk_cols = np.array(qk_cols)
        v_cols = []
        for p in range(4):
            ha, hb = heads[2 * p], heads[2 * p + 1]
            v_cols.extend(range(ha * 192 + 128, ha * 192 + 192))
            v_cols.extend(range(hb * 192 + 128, hb * 192 + 192))
        v_cols = np.array(v_cols)
        in_maps.append(
            {
                "xt": np.ascontiguousarray(x[b].T, dtype=np.float32),
                "wqk": np.ascontiguousarray(W_qkv[:, qk_cols], dtype=np.float32),
                "bqk": np.ascontiguousarray(b_qkv[qk_cols], dtype=np.float32),
                "wv": np.ascontiguousarray(W_qkv[:, v_cols], dtype=np.float32),
                "bv": np.ascontiguousarray(
                    b_qkv[v_cols].reshape(1, 512), dtype=np.float32
                ),
                "wproj": np.ascontiguousarray(
                    W_proj[hg * 512 : (hg + 1) * 512, :], dtype=np.float32
                ),
            }
        )
    return in_maps


_NC = None


def kernel(x, W_qkv, b_qkv, W_proj, b_proj, _trace=False):
    global _NC
    x = np.asarray(x, dtype=np.float32)
    W_qkv = np.asarray(W_qkv, dtype=np.float32)
    b_qkv = np.asarray(b_qkv, dtype=np.float32)
    W_proj = np.asarray(W_proj, dtype=np.float32)
    b_proj = np.asarray(b_proj, dtype=np.float32)

    in_maps = _shard_inputs(x, W_qkv, b_qkv, W_proj)
    if _NC is None:
        _NC = build_kernel()
    res = run_bass_kernel_spmd(
        _NC, in_maps, core_ids=list(range(8)), trace=_trace,
        trace_cores=list(range(8)) if _trace else None,
    )
    out = np.empty((B, T, C), dtype=np.float32)
    for b in range(B):
        out[b] = res.results[2 * b]["y"] + res.results[2 * b + 1]["y"] + b_proj
    if _trace:
        return out, res
    return out
